# revision 1
# baseline (speedup 1.0000x reference)
"""MLA forward Bass kernel for 8 TRN2 NeuronCores.

Sharding: pure query-row sharding. Core c handles batch b = c//4 and query rows
[sl*512, (sl+1)*512) with sl = c%4, for ALL 16 heads. Keys/values span the full
sequence, so the compressed-KV path (kvc, k_rope) is computed per-core for the
whole batch (replicated across the 4 cores that share a batch), while the Q
path, attention, and the output projection only cover the core's 512 query
rows. The full output-projection contraction (all 16 heads) is local, so no
cross-core reduction is needed: the host just concatenates the 8 row-blocks.

Layouts: everything TensorE-facing is kept transposed ([feature, seq]) so the
feature dim sits on partitions and matmuls contract over it. Softmax runs on
S^T tiles [k, q]: exp on ACT (no max-shift; scores are O(1) here), denominator
via DVE tile-adds + a ones-matmul partition reduction, normalization folded
into the PSUM->SBUF drain of the attention output. RoPE's rotate-half is a
constant 128x128 permutation matmul.

Perf notes (measured on HW via NTFF traces; baseline 1,197,956 ns):
- All big matmul operands are bf16: same PE row rate as f32r at free>=256,
  but LDWEIGHTS runs at 1 cyc/row with compiler-auto FWL (f32 is 2 cyc/row
  and FWL-ineligible) and weight/x DMA bytes halve. PSUM accum stays fp32;
  rel_fro vs the fp32 reference is ~5e-3 (budget 2e-2).
- Attention processes head PAIRS with softmax normalization deferred by one
  pair and split in two stages: the den reductions (_norm_den) queue behind
  the next pair's decompress matmuls, the broadcast+scale (_norm_fin) lands
  after the rope matmuls so the DVE reciprocal latency is PE-covered. The
  den1/bc tiles live in the st_ps ring, whose previous tenants (score tiles)
  are already drained by exp — the aux ring's tenant (rope rot, slow DVE
  consumer) stalled ~2.4us per pair, and a wk_ps variant stalled the next
  pair's decompress (see below).
- Phase 0 drains compress PSUM to raw bf16 tiles and defers each block's
  rmsnorm/rope postprocessing behind the next block's matmuls.
- wo streams once per (head, block) serving all 4 query tiles; wckv/wkr are
  held in SBUF across seq blocks; prologue weight DMAs interleave with the
  first block's x tiles (first matmul ~14us in instead of ~36us); each
  group's wdv tiles fetch before wdk (v is consumed first); wo prefetch is
  8 tiles deep.
- Tried and reverted: seq-sharding the kv compress via a 4-core DRAM
  AllGather (kernel_v3_ag.py.bak) — the collective costs ~90us on this
  axon path, washing out the ~58us of saved PE rows; aux_ps=2/st_ps=2
  pool sizing; and routing the normalize split through the wk_ps ring
  instead of st_ps (kernel_v5/v6/v8 baks) — wk tenants are drained by ACT
  copies that queue behind the kt-loop exp backlog, stalling the next
  pair's decompress worse than the aux-buffer stall it removed (+26us both
  times measured). Note the device clock state adds
  ~±4% run-to-run (HAM/P0 downclock under sustained load drifts
  per-matmul time 273->321ns), so only deltas beyond that are trusted.
"""

import os
import sys

for _p in ("/root/.axon_site/_ro/trn_rl_repo", "/opt/trn_rl_repo"):
    if os.path.isdir(_p) and _p not in sys.path:
        sys.path.insert(0, _p)

import numpy as np

import concourse.bass as bass
import concourse.tile as tile
from concourse import mybir
from concourse.bass_utils import run_bass_kernel_spmd

F32 = mybir.dt.float32
F32R = mybir.dt.float32r
BF16 = mybir.dt.bfloat16

D = 2048        # d_model
S = 2048        # seq len
B = 2           # batch
H = 16          # heads
HD = 128        # nope head dim
KV = 512        # kv lora rank
QL = 768        # q lora rank
RD = 64         # rope dim
EPS = 1e-6
SQ = 512        # query rows per core
N_CORES = 8
GROUPS = 4      # head groups of 4
GH = 4          # heads per group
SCALE = 1.0 / float(np.sqrt(HD + RD))

NKV = KV // 128   # 4 kv-lora chunks
NQL = QL // 128   # 6 q-lora chunks
NS = S // 512     # 4 seq blocks
NST = S // 128    # 16 seq tiles


# ---------------------------------------------------------------------------
# The walrus build in this container only encodes a single sync-wait on a
# Drain (TPB_CTRL) instruction, but TileContext._drain_and_barrier parks the
# whole global-clock wait set on the tail drain ("Too many sync wait
# commands"). Hoist the waits onto single-wait NOPs ahead of a bare drain.
def _patch_tile_drain():
    from bass_rust import ScopedClock

    def _drain_and_barrier(self, tick_clock, wait_clock):
        probe = self.nc.sync.nop(nofuse=True)
        wait_clock.add_sem_waits(
            probe.ins, ScopedClock({None: tick_clock.global_clock})
        )
        si = probe.ins.sync_info
        waits = list(si.on_wait) if si is not None else []
        if len(waits) > 1:
            probe.ins.sync_info = mybir.SyncInfo(on_wait=waits[:1], on_update=[])
            for w in waits[1:]:
                extra = self.nc.sync.nop(nofuse=True)
                extra.ins.sync_info = mybir.SyncInfo(on_wait=[w], on_update=[])
        self.nc.sync.drain()

        self.nc.all_engine_barrier()
        assert self.sems is not None
        popped = self.nc._tile_sem_poison_stack.pop()
        assert popped is self._sem_poison
        self.nc.clear_and_free_semaphores(list(self.sems.allocated().values()))
        self.nc.all_engine_barrier()

    tile.TileContext._drain_and_barrier = _drain_and_barrier


_patch_tile_drain()


def _r(ap):
    return ap.bitcast(F32R)


def build_nc():
    nc = bass.Bass()

    xT = nc.dram_tensor("xT", [D, S], BF16, kind="ExternalInput")
    xqT = nc.dram_tensor("xqT", [D, SQ], BF16, kind="ExternalInput")
    wcq = nc.dram_tensor("wcq", [D, QL], BF16, kind="ExternalInput")
    wckv = nc.dram_tensor("wckv", [D, KV], BF16, kind="ExternalInput")
    wkr2 = nc.dram_tensor("wkr2", [D, 128], BF16, kind="ExternalInput")
    wdq = nc.dram_tensor("wdq", [QL, H * HD], BF16, kind="ExternalInput")
    wdqr = nc.dram_tensor("wdqr", [QL, H * RD], BF16, kind="ExternalInput")
    wdk = nc.dram_tensor("wdk", [KV, H * HD], BF16, kind="ExternalInput")
    wdv = nc.dram_tensor("wdv", [KV, H * HD], BF16, kind="ExternalInput")
    wo = nc.dram_tensor("wo", [H * HD, D], BF16, kind="ExternalInput")
    gq = nc.dram_tensor("gq", [1, QL], F32, kind="ExternalInput")
    gkv = nc.dram_tensor("gkv", [1, KV], F32, kind="ExternalInput")
    cosk = nc.dram_tensor("cosk", [128, S], F32, kind="ExternalInput")
    sink = nc.dram_tensor("sink", [128, S], F32, kind="ExternalInput")
    cosq = nc.dram_tensor("cosq", [128, SQ], F32, kind="ExternalInput")
    sinq = nc.dram_tensor("sinq", [128, SQ], F32, kind="ExternalInput")
    rotp = nc.dram_tensor("rotp", [128, 128], BF16, kind="ExternalInput")
    out = nc.dram_tensor("out", [SQ, D], F32, kind="ExternalOutput")
    debug = bool(int(os.environ.get("MLA_DEBUG", "0")))
    if debug:
        dbg_kvcT = nc.dram_tensor("dbg_kvcT", [KV, S], BF16, kind="ExternalOutput")
        dbg_qcT = nc.dram_tensor("dbg_qcT", [QL, SQ], BF16, kind="ExternalOutput")
        dbg_krT = nc.dram_tensor("dbg_krT", [128, S], BF16, kind="ExternalOutput")
        dbg_oT = nc.dram_tensor("dbg_oT", [H * 128, SQ], BF16, kind="ExternalOutput")

    with tile.TileContext(nc) as tc:
        _build_body(nc, tc, locals(), debug)
    _split_excess_waits(nc)
    return nc


# This walrus build encodes at most one sync-wait per engine instruction;
# hoist surplus waits onto single-wait NOPs right before the instruction on
# the same engine queue (in-order execution keeps the semantics identical).
def _split_excess_waits(nc, max_waits=1):
    n_nops = 0
    for f in nc.m.functions:
        for bb in f.blocks:
            out = []
            for ins in bb.instructions:
                si = ins.sync_info
                if si is not None:
                    sem = [w for w in si.on_wait if w.sync_type == "semaphore"]
                    other = [w for w in si.on_wait if w.sync_type != "semaphore"]
                    budget = max(max_waits - len(other), 0)
                    if len(sem) > budget:
                        extra, keep = sem[:-budget] if budget else sem, (
                            sem[-budget:] if budget else [])
                        for j, w in enumerate(extra):
                            nop = mybir.InstNoOp(
                                name=f"{ins.name}-wsplit{j}",
                                engine=ins.engine,
                                bass_nofuse=True,
                                sync_info=mybir.SyncInfo(
                                    on_wait=[w], on_update=[]),
                            )
                            out.append(nop)
                            n_nops += 1
                        ins.sync_info = mybir.SyncInfo(
                            on_wait=other + keep,
                            on_update=list(si.on_update))
                out.append(ins)
            bb.instructions = out
    return n_nops


def _norm_den(nc, misc, st_ps, ones128, pending):
    """Stage 1 of softmax normalization: den1 = sum_partitions(den) via a
    ones-matmul into row 0 of an st_ps tile. The st ring's previous tenants
    are score tiles already drained by exp, so unlike the aux ring (whose
    tenant is the rope rot tile with a slow DVE consumer) or the wk ring
    (whose tenants gate the next pair's decompress), nothing stalls."""
    recs = []
    for h, pot, den in pending:
        dst = st_ps.tile([128, SQ], F32, tag="st", name="dst")
        nc.tensor.matmul(dst[0:1, :], _r(ones128), _r(den))
        rec = misc.tile([1, SQ], F32R, tag="rec")
        nc.scalar.copy(rec, dst[0:1, :])
        with nc.allow_low_precision(reason="f32r is full fp32 bits"):
            nc.vector.reciprocal(rec, rec)
        recs.append(rec)
    return recs


def _norm_fin(nc, misc, st_ps, oT, ones1, pending, recs):
    """Stage 2: broadcast 1/den across partitions and scale the attention
    output into oT[h]. Emitted after the rope matmuls so the DVE reciprocal
    latency is hidden behind PE work."""
    for (h, pot, den), rec in zip(pending, recs):
        bc = st_ps.tile([128, SQ], F32, tag="st", name="bcst")
        nc.tensor.matmul(bc, _r(ones1), _r(rec))
        bc_s = misc.tile([128, SQ], F32, tag="bcs")
        nc.scalar.copy(bc_s, bc)
        nc.vector.tensor_mul(oT[h], pot, bc_s)


def _build_body(nc, tc, t, debug=False):
    from contextlib import ExitStack

    ctx = ExitStack()
    with ctx:
        consts = ctx.enter_context(tc.tile_pool(name="consts", bufs=1))
        persist = ctx.enter_context(tc.tile_pool(name="persist", bufs=1))
        misc = ctx.enter_context(tc.tile_pool(name="misc", bufs=2))
        # PSUM pools: aux first (lives through whole kernel), then phase pools.
        aux_ps = ctx.enter_context(tc.tile_pool(name="aux_ps", bufs=1, space="PSUM"))

        # ---- constants -----------------------------------------------------
        ones128f = consts.tile([128, 1], F32)
        nc.vector.memset(ones128f, 1.0)
        ones128 = consts.tile([128, 1], F32R)
        nc.scalar.copy(ones128, ones128f)
        ones1f = consts.tile([1, 128], F32)
        nc.vector.memset(ones1f, 1.0)
        ones1 = consts.tile([1, 128], F32R)
        nc.scalar.copy(ones1, ones1f)
        gq_s = consts.tile([1, QL], F32R)
        nc.sync.dma_start(out=gq_s, in_=t["gq"][:, :].bitcast(F32R))
        gkv_s = consts.tile([1, KV], F32R)
        nc.sync.dma_start(out=gkv_s, in_=t["gkv"][:, :].bitcast(F32R))
        eps_s = consts.tile([1, 1], F32)
        nc.vector.memset(eps_s, EPS)
        rotp_s = consts.tile([128, 128], BF16)
        nc.sync.dma_start(out=rotp_s, in_=t["rotp"][:, :])
        cosq_s = consts.tile([128, SQ], F32)
        nc.sync.dma_start(out=cosq_s, in_=t["cosq"][:, :])
        sinq_s = consts.tile([128, SQ], F32)
        nc.sync.dma_start(out=sinq_s, in_=t["sinq"][:, :])

        # ---- persistent tiles (bf16: matmul operands -> FWL weight loads) --
        kvcT = [persist.tile([128, S], BF16, tag=f"kvcT{c}", name=f"kvcT{c}") for c in range(NKV)]
        krT = persist.tile([128, S], BF16, tag="krT")
        qcT = [persist.tile([128, SQ], BF16, tag=f"qcT{c}", name=f"qcT{c}") for c in range(NQL)]
        oT = [persist.tile([128, SQ], BF16, tag=f"oT{h}", name=f"oT{h}") for h in range(H)]

        # ===================================================================
        # Phase 0: compress. kvcT/krT over full seq, qcT over own query rows.
        # ===================================================================
        with nc.named_scope("p0_compress", notify=True), \
             tc.tile_pool(name="misc0", bufs=2) as misc0, \
             tc.tile_pool(name="xtp", bufs=3) as xtp, \
             tc.tile_pool(name="wkvhold", bufs=1) as wkvhold, \
             tc.tile_pool(name="wstream", bufs=3) as wstream, \
             tc.tile_pool(name="acc_ps", bufs=6, space="PSUM") as acc_ps:
            # wckv/wkr are reused by all 4 seq blocks: load once, keep in SBUF
            wkv_h = [wkvhold.tile([128, KV], BF16, tag=f"wckv{d}", name=f"wckv{d}")
                     for d in range(16)]
            wkr_h = [wkvhold.tile([128, 128], BF16, tag=f"wkr{d}", name=f"wkr{d}")
                     for d in range(16)]
            def p0_post(scol, kvraw, kraw):
                # rmsnorm over kv features (partition dim across the 4 chunks)
                ssq = aux_ps.tile([1, 512], F32, tag="aux")
                for c in range(NKV):
                    sq = misc0.tile([128, 512], F32R, tag="sq")
                    nc.scalar.square(sq, kvraw[c])
                    nc.tensor.matmul(ssq, _r(ones128), _r(sq),
                                     start=(c == 0), stop=(c == NKV - 1))
                rstd = misc0.tile([1, 512], F32R, tag="rstd")
                nc.scalar.activation(rstd, ssq,
                                     mybir.ActivationFunctionType.Sqrt,
                                     bias=eps_s[:, :], scale=1.0 / KV)
                with nc.allow_low_precision(reason="f32r is full fp32 bits"):
                    nc.vector.reciprocal(rstd, rstd)
                for c in range(NKV):
                    bc = aux_ps.tile([128, 512], F32, tag="aux")
                    nc.tensor.matmul(
                        bc, _r(gkv_s[:, c * 128:(c + 1) * 128]), _r(rstd))
                    bc_s = misc.tile([128, 512], F32, tag="bcs")
                    nc.scalar.copy(bc_s, bc)
                    nc.vector.tensor_mul(kvcT[c][:, scol], kvraw[c], bc_s)

                # rope on the (duplicated-rows) k_rope block
                ck = misc0.tile([128, 512], F32, tag="ck")
                nc.sync.dma_start(out=ck, in_=t["cosk"][:, scol])
                sk = misc0.tile([128, 512], F32, tag="sk")
                nc.sync.dma_start(out=sk, in_=t["sink"][:, scol])
                rot = aux_ps.tile([128, 512], F32, tag="aux")
                nc.tensor.matmul(rot, rotp_s, kraw)
                t1 = misc0.tile([128, 512], F32, tag="ropet1")
                nc.vector.tensor_mul(t1, kraw, ck)
                t2 = misc0.tile([128, 512], F32, tag="ropet2")
                nc.vector.tensor_mul(t2, rot, sk)
                nc.vector.tensor_add(krT[:, scol], t1, t2)

            p0_pending = None
            for sb in range(NS):
                scol = slice(sb * 512, (sb + 1) * 512)
                pkv = [acc_ps.tile([128, 512], F32, tag="acc", name="pkv") for _ in range(NKV)]
                pkr = acc_ps.tile([128, 512], F32, tag="acc")
                for d in range(16):
                    drow = slice(d * 128, (d + 1) * 128)
                    if sb == 0:
                        nc.sync.dma_start(out=wkv_h[d], in_=t["wckv"][drow, :])
                        nc.sync.dma_start(out=wkr_h[d], in_=t["wkr2"][drow, :])
                    xt = xtp.tile([128, 512], BF16, tag="xt")
                    nc.sync.dma_start(out=xt, in_=t["xT"][drow, scol])
                    for c in range(NKV):
                        nc.tensor.matmul(
                            pkv[c], wkv_h[d][:, c * 128:(c + 1) * 128], xt,
                            start=(d == 0), stop=(d == 15))
                    nc.tensor.matmul(pkr, wkr_h[d], xt,
                                     start=(d == 0), stop=(d == 15))
                # drain psum to raw bf16 sbuf tiles (releases acc banks), then
                # run the PREVIOUS block's normalize behind this block's MMs.
                kvraw = [misc0.tile([128, 512], BF16, tag=f"kvraw{c}",
                                    name=f"kvraw{c}")
                         for c in range(NKV)]
                for c in range(NKV):
                    nc.scalar.copy(kvraw[c], pkv[c])
                kraw = misc0.tile([128, 512], BF16, tag="kraw")
                nc.scalar.copy(kraw, pkr)
                if p0_pending is not None:
                    p0_post(*p0_pending)
                p0_pending = (scol, kvraw, kraw)

            # qcT over own query rows
            pqc = [acc_ps.tile([128, 512], F32, tag="acc", name="pqc") for _ in range(NQL)]
            for d in range(16):
                drow = slice(d * 128, (d + 1) * 128)
                xt = xtp.tile([128, 512], BF16, tag="xt")
                nc.sync.dma_start(out=xt, in_=t["xqT"][drow, :])
                wq_t = wstream.tile([128, QL], BF16, tag="wcq")
                nc.sync.dma_start(out=wq_t, in_=t["wcq"][drow, :])
                for c in range(NQL):
                    nc.tensor.matmul(
                        pqc[c], wq_t[:, c * 128:(c + 1) * 128], xt,
                        start=(d == 0), stop=(d == 15))
            if p0_pending is not None:
                p0_post(*p0_pending)
                p0_pending = None
            ssq = aux_ps.tile([1, 512], F32, tag="aux")
            for c in range(NQL):
                sq = misc0.tile([128, 512], F32R, tag="sq")
                nc.scalar.square(sq, pqc[c])
                nc.tensor.matmul(ssq, _r(ones128), _r(sq),
                                 start=(c == 0), stop=(c == NQL - 1))
            rstd = misc0.tile([1, 512], F32R, tag="rstd")
            nc.scalar.activation(rstd, ssq, mybir.ActivationFunctionType.Sqrt,
                                 bias=eps_s[:, :], scale=1.0 / QL)
            with nc.allow_low_precision(reason="f32r is full fp32 bits"):
                nc.vector.reciprocal(rstd, rstd)
            for c in range(NQL):
                bc = aux_ps.tile([128, 512], F32, tag="aux")
                nc.tensor.matmul(bc, _r(gq_s[:, c * 128:(c + 1) * 128]), _r(rstd))
                bc_s = misc.tile([128, 512], F32, tag="bcs")
                nc.scalar.copy(bc_s, bc)
                nc.vector.tensor_mul(qcT[c], pqc[c], bc_s)

        # ===================================================================
        # Phase A: per head group -- decompress k/v/q, attention.
        # ===================================================================
        with nc.named_scope("pA_attn", notify=True), \
             tc.tile_pool(name="vpool", bufs=24) as vpool, \
             tc.tile_pool(name="khp", bufs=2) as khp, \
             tc.tile_pool(name="qnp", bufs=2) as qnp, \
             tc.tile_pool(name="qrp", bufs=2) as qrp, \
             tc.tile_pool(name="ptp", bufs=4) as ptp, \
             tc.tile_pool(name="denp", bufs=2) as denp, \
             tc.tile_pool(name="wdqp", bufs=6) as wdqp, \
             tc.tile_pool(name="wdqrp", bufs=6) as wdqrp, \
             tc.tile_pool(name="wdkp", bufs=4) as wdkp, \
             tc.tile_pool(name="wdvp", bufs=4) as wdvp, \
             tc.tile_pool(name="st_ps", bufs=3, space="PSUM") as st_ps, \
             tc.tile_pool(name="ot_ps", bufs=2, space="PSUM") as ot_ps, \
             tc.tile_pool(name="wk_ps", bufs=2, space="PSUM") as wk_ps:

            pending = []
            for g in range(GROUPS):
                gcol = slice(g * 512, (g + 1) * 512)
                # stream this group's decompress weights
                wdv_t = [wdvp.tile([128, 512], BF16, tag="wdv", name="wdv_t") for _ in range(NKV)]
                for c in range(NKV):
                    nc.sync.dma_start(
                        out=wdv_t[c], in_=t["wdv"][c * 128:(c + 1) * 128, gcol])
                wdk_t = [wdkp.tile([128, 512], BF16, tag="wdk", name="wdk_t") for _ in range(NKV)]
                for c in range(NKV):
                    nc.sync.dma_start(
                        out=wdk_t[c], in_=t["wdk"][c * 128:(c + 1) * 128, gcol])
                wdq_t = [wdqp.tile([128, 512], BF16, tag="wdq", name="wdq_t") for _ in range(NQL)]
                for c in range(NQL):
                    nc.sync.dma_start(
                        out=wdq_t[c], in_=t["wdq"][c * 128:(c + 1) * 128, gcol])
                grcol = slice(g * 256, (g + 1) * 256)
                wdqr_t = [wdqrp.tile([128, 256], BF16, tag="wdqr", name="wdqr_t") for _ in range(NQL)]
                for c in range(NQL):
                    nc.sync.dma_start(
                        out=wdqr_t[c], in_=t["wdqr"][c * 128:(c + 1) * 128, grcol])

                # v for all 4 heads of the group: moving = wdv (512 wide),
                # stationary = kvc seq-tile. Halves the matmul/LDW count vs
                # the per-pair 256-wide variant.
                vt = {}
                for st in range(NST):
                    pv = wk_ps.tile([128, 512], F32, tag="wk")
                    for c in range(NKV):
                        nc.tensor.matmul(
                            pv, kvcT[c][:, st * 128:(st + 1) * 128], wdv_t[c],
                            start=(c == 0), stop=(c == NKV - 1))
                    v_s = vpool.tile([128, 512], BF16, tag="v")
                    nc.scalar.copy(v_s, pv)
                    vt[st] = v_s

                for pair in range(GH // 2):
                    hA = g * GH + 2 * pair
                    hB = hA + 1
                    colA = slice((2 * pair) * 128, (2 * pair + 1) * 128)
                    colB = slice((2 * pair + 1) * 128, (2 * pair + 2) * 128)

                    # k_nope^T for both heads: [128 d, S]
                    khA = khp.tile([128, S], BF16, tag="kh")
                    khB = khp.tile([128, S], BF16, tag="kh")
                    for kh, hcol in ((khA, colA), (khB, colB)):
                        for blk in range(NS):
                            bcol = slice(blk * 512, (blk + 1) * 512)
                            pk = wk_ps.tile([128, 512], F32, tag="wk")
                            for c in range(NKV):
                                nc.tensor.matmul(
                                    pk, wdk_t[c][:, hcol], kvcT[c][:, bcol],
                                    start=(c == 0), stop=(c == NKV - 1))
                            nc.scalar.copy(kh[:, bcol], pk)

                    # q_nope^T for both heads: [128 d, SQ]
                    qnA = qnp.tile([128, SQ], BF16, tag="qn")
                    qnB = qnp.tile([128, SQ], BF16, tag="qn")
                    for qn, hcol in ((qnA, colA), (qnB, colB)):
                        pq = wk_ps.tile([128, SQ], F32, tag="wk")
                        for c in range(NQL):
                            nc.tensor.matmul(pq, wdq_t[c][:, hcol], qcT[c],
                                             start=(c == 0), stop=(c == NQL - 1))
                        nc.scalar.copy(qn, pq)

                    # normalize the PREVIOUS pair, stage 1 (its den DVE
                    # chain drained behind the decompress matmuls above)
                    recs = _norm_den(nc, misc, st_ps, ones128, pending)

                    # q_rope for the pair (two heads stacked on partitions)
                    prcol = slice(pair * 128, (pair + 1) * 128)
                    pqr = wk_ps.tile([128, SQ], F32, tag="wk")
                    for c in range(NQL):
                        nc.tensor.matmul(
                            pqr, wdqr_t[c][:, prcol], qcT[c],
                            start=(c == 0), stop=(c == NQL - 1))
                    qraw = misc.tile([128, SQ], BF16, tag="qraw")
                    nc.scalar.copy(qraw, pqr)
                    rot = aux_ps.tile([128, SQ], F32, tag="aux")
                    nc.tensor.matmul(rot, rotp_s, qraw)
                    t1 = misc.tile([128, SQ], F32, tag="ropet1")
                    nc.vector.tensor_mul(t1, qraw, cosq_s)
                    t2 = misc.tile([128, SQ], F32, tag="ropet2")
                    nc.vector.tensor_mul(t2, rot, sinq_s)
                    qr_roped = qrp.tile([128, SQ], BF16, tag="qr")
                    nc.vector.tensor_add(qr_roped, t1, t2)

                    # ...stage 2 lands after the rope matmuls so the DVE
                    # reciprocal latency is PE-covered.
                    _norm_fin(nc, misc, st_ps, oT, ones1, pending, recs)
                    pending.clear()

                    # attention for the pair. The two K=64 rope matmuls run on
                    # disjoint PE row-groups (base_partition 0 / 64) and
                    # overlap; stops are interleaved so exp can chase.
                    potA = ot_ps.tile([128, SQ], F32, tag="ot")
                    potB = ot_ps.tile([128, SQ], F32, tag="ot")
                    denA = denp.tile([128, SQ], F32R, tag="den")
                    denB = denp.tile([128, SQ], F32R, tag="den")
                    for kt in range(NST):
                        kcol = slice(kt * 128, (kt + 1) * 128)
                        pstA = st_ps.tile([128, SQ], F32, tag="st")
                        pstB = st_ps.tile([128, SQ], F32, tag="st")
                        nc.tensor.matmul(pstA, khA[:, kcol], qnA,
                                         start=True, stop=False)
                        nc.tensor.matmul(pstB, khB[:, kcol], qnB,
                                         start=True, stop=False)
                        nc.tensor.matmul(pstA, krT[0:64, kcol],
                                         qr_roped[0:64, :],
                                         start=False, stop=True)
                        nc.tensor.matmul(pstB, krT[64:128, kcol],
                                         qr_roped[64:128, :],
                                         start=False, stop=True)
                        ptA = ptp.tile([128, SQ], BF16, tag="pt")
                        nc.scalar.activation(ptA, pstA,
                                             mybir.ActivationFunctionType.Exp,
                                             scale=SCALE)
                        ptB = ptp.tile([128, SQ], BF16, tag="pt")
                        nc.scalar.activation(ptB, pstB,
                                             mybir.ActivationFunctionType.Exp,
                                             scale=SCALE)
                        if kt == 0:
                            nc.vector.tensor_copy(denA, ptA)
                            nc.vector.tensor_copy(denB, ptB)
                        else:
                            nc.vector.tensor_add(denA, denA, ptA)
                            nc.vector.tensor_add(denB, denB, ptB)
                        vs = vt[kt]
                        nc.tensor.matmul(
                            potA, vs[:, colA], ptA,
                            start=(kt == 0), stop=(kt == NST - 1))
                        nc.tensor.matmul(
                            potB, vs[:, colB], ptB,
                            start=(kt == 0), stop=(kt == NST - 1))

                    pending.append((hA, potA, denA))
                    pending.append((hB, potB, denB))

            recs = _norm_den(nc, misc, st_ps, ones128, pending)
            _norm_fin(nc, misc, st_ps, oT, ones1, pending, recs)
            pending.clear()

        if debug:
            for c in range(NKV):
                nc.sync.dma_start(
                    out=t["dbg_kvcT"][c * 128:(c + 1) * 128, :], in_=kvcT[c])
            for c in range(NQL):
                nc.sync.dma_start(
                    out=t["dbg_qcT"][c * 128:(c + 1) * 128, :], in_=qcT[c])
            nc.sync.dma_start(out=t["dbg_krT"][:, :], in_=krT)
            for h in range(H):
                nc.sync.dma_start(
                    out=t["dbg_oT"][h * 128:(h + 1) * 128, :], in_=oT[h])

        # ===================================================================
        # Phase B: output projection, all 16 heads, PSUM-accumulated.
        # Loop order: wo tile loads once per (h, blk) and serves all 4 query
        # tiles (wo HBM traffic 16MB instead of 64MB).
        # ===================================================================
        NQT = SQ // 128
        with nc.named_scope("pB_outproj", notify=True), \
             tc.tile_pool(name="wop", bufs=8) as wop, \
             tc.tile_pool(name="outs", bufs=4) as outs, \
             tc.tile_pool(name="po_ps", bufs=4, space="PSUM") as po_ps:
            for blk in range(NS):
                bcol = slice(blk * 512, (blk + 1) * 512)
                po = [po_ps.tile([128, 512], F32, tag="po", name=f"po{qt}")
                      for qt in range(NQT)]
                for h in range(H):
                    wo_t = wop.tile([128, 512], BF16, tag="wo")
                    nc.sync.dma_start(
                        out=wo_t, in_=t["wo"][h * 128:(h + 1) * 128, bcol])
                    for qt in range(NQT):
                        nc.tensor.matmul(
                            po[qt], oT[h][:, qt * 128:(qt + 1) * 128], wo_t,
                            start=(h == 0), stop=(h == H - 1))
                for qt in range(NQT):
                    o_s = outs.tile([128, 512], F32, tag="os")
                    nc.scalar.copy(o_s, po[qt])
                    nc.sync.dma_start(
                        out=t["out"][qt * 128:(qt + 1) * 128, bcol], in_=o_s)


_NC_CACHE = None


def _get_nc():
    global _NC_CACHE
    if _NC_CACHE is None:
        _NC_CACHE = build_nc()
    return _NC_CACHE


def _rope_tables(positions):
    """cos/sin tables in transposed-packed layout [128, len(positions)]:
    rows 0:64 and 64:128 both hold the [RD, s] table (two rope vectors are
    stacked per 128 partitions)."""
    inv_freq = 1.0 / (10000.0 ** (np.arange(0, RD, 2, dtype=np.float32) / RD))
    ang = positions[:, None].astype(np.float32) * inv_freq[None, :]  # [s, 32]
    cos = np.concatenate([np.cos(ang), np.cos(ang)], axis=-1)        # [s, 64]
    sin = np.concatenate([np.sin(ang), np.sin(ang)], axis=-1)
    cosT = np.ascontiguousarray(cos.T)                               # [64, s]
    sinT = np.ascontiguousarray(sin.T)
    return (np.concatenate([cosT, cosT], axis=0),
            np.concatenate([sinT, sinT], axis=0))


def _rot_perm():
    m = np.zeros((128, 128), dtype=np.float32)
    for b0 in (0, 64):
        for i in range(32):
            m[b0 + i + 32, b0 + i] = -1.0   # rot[m] = -t[m+32], m < 32
            m[b0 + i, b0 + i + 32] = 1.0    # rot[m] = +t[m-32], m >= 32
    return m


def kernel(x, Wcq, g_q, Wdq, Wdqr, Wckv, g_kv, Wdk, Wdv, Wkr, Wo):
    import ml_dtypes

    bf16 = ml_dtypes.bfloat16
    nc = _get_nc()

    x = np.asarray(x, dtype=np.float32)
    xT = [np.ascontiguousarray(x[b].T).astype(bf16) for b in range(B)]  # [D, S]
    wkr2 = np.ascontiguousarray(
        np.concatenate([Wkr, Wkr], axis=1)).astype(bf16)  # [D, 128]
    cosk, sink = _rope_tables(np.arange(S))
    rotp = _rot_perm().astype(bf16)

    shared = {
        "wcq": np.ascontiguousarray(Wcq).astype(bf16),
        "wckv": np.ascontiguousarray(Wckv).astype(bf16),
        "wkr2": wkr2,
        "wdq": np.ascontiguousarray(Wdq).astype(bf16),
        "wdqr": np.ascontiguousarray(Wdqr).astype(bf16),
        "wdk": np.ascontiguousarray(Wdk).astype(bf16),
        "wdv": np.ascontiguousarray(Wdv).astype(bf16),
        "wo": np.ascontiguousarray(Wo).astype(bf16),
        "gq": np.ascontiguousarray(g_q, dtype=np.float32).reshape(1, QL),
        "gkv": np.ascontiguousarray(g_kv, dtype=np.float32).reshape(1, KV),
        "cosk": np.ascontiguousarray(cosk),
        "sink": np.ascontiguousarray(sink),
        "rotp": rotp,
    }

    in_maps = []
    for core in range(N_CORES):
        b, sl = core // 4, core % 4
        rows = np.arange(sl * SQ, (sl + 1) * SQ)
        cq, sq_t = _rope_tables(rows)
        m = dict(shared)
        m["xT"] = xT[b]
        m["xqT"] = np.ascontiguousarray(xT[b][:, sl * SQ:(sl + 1) * SQ])
        m["cosq"] = np.ascontiguousarray(cq)
        m["sinq"] = np.ascontiguousarray(sq_t)
        in_maps.append(m)

    trace = bool(int(os.environ.get("MLA_TRACE", "0")))
    res = run_bass_kernel_spmd(
        nc, in_maps, core_ids=list(range(N_CORES)), trace=trace,
        trace_cores=list(range(N_CORES)) if trace else None,
        stitch_traces=bool(int(os.environ.get("MLA_STITCH", "0"))),
        tmpdir=os.environ.get("MLA_TMPDIR") or None,
    )
    kernel.last_result = res

    out = np.empty((B, S, D), dtype=np.float32)
    for core in range(N_CORES):
        b, sl = core // 4, core % 4
        out[b, sl * SQ:(sl + 1) * SQ, :] = res.results[core]["out"]
    return out



# revision 6
# speedup vs baseline: 1.2487x; 1.2487x over previous
"""MLA forward Bass kernel for 8 TRN2 NeuronCores.

Sharding: pure query-row sharding. Core c handles batch b = c//4 and query rows
[sl*512, (sl+1)*512) with sl = c%4, for ALL 16 heads. Keys/values span the full
sequence, so the compressed-KV path (kvc, k_rope) is computed per-core for the
whole batch (replicated across the 4 cores that share a batch), while the Q
path, attention, and the output projection only cover the core's 512 query
rows. The full output-projection contraction (all 16 heads) is local, so no
cross-core reduction is needed: the host just concatenates the 8 row-blocks.

Layouts: everything TensorE-facing is kept transposed ([feature, seq]) so the
feature dim sits on partitions and matmuls contract over it. Softmax runs on
S^T tiles [k, q]: exp on ACT (no max-shift; scores are O(1) here), denominator
via DVE tile-adds + a ones-matmul partition reduction, normalization folded
into the PSUM->SBUF drain of the attention output. RoPE's rotate-half is a
constant 128x128 permutation matmul.

Perf notes (measured on HW via NTFF traces; baseline 1,197,956 ns):
- All big matmul operands are bf16: same PE row rate as f32r at free>=256,
  but LDWEIGHTS runs at 1 cyc/row with compiler-auto FWL (f32 is 2 cyc/row
  and FWL-ineligible) and weight/x DMA bytes halve. PSUM accum stays fp32;
  rel_fro vs the fp32 reference is ~5e-3 (budget 2e-2).
- Attention processes head PAIRS with softmax normalization deferred by one
  pair and split in two stages: the den reductions (_norm_den) queue behind
  the next pair's decompress matmuls, the broadcast+scale (_norm_fin) lands
  after the rope matmuls so the DVE reciprocal latency is PE-covered. The
  den1/bc tiles live in the st_ps ring, whose previous tenants (score tiles)
  are already drained by exp — the aux ring's tenant (rope rot, slow DVE
  consumer) stalled ~2.4us per pair, and a wk_ps variant stalled the next
  pair's decompress (see below).
- Phase 0 drains compress PSUM to raw bf16 tiles and defers each block's
  rmsnorm/rope postprocessing behind the next block's matmuls.
- wo streams once per (head, block) serving all 4 query tiles; wckv/wkr are
  held in SBUF across seq blocks; prologue weight DMAs interleave with the
  first block's x tiles (first matmul ~14us in instead of ~36us); each
  group's wdv tiles fetch before wdk (v is consumed first); wo prefetch is
  8 tiles deep.
- Tried and reverted: seq-sharding the kv compress via a 4-core DRAM
  AllGather (kernel_v3_ag.py.bak) — the collective costs ~90us on this
  axon path, washing out the ~58us of saved PE rows; aux_ps=2/st_ps=2
  pool sizing; and routing the normalize split through the wk_ps ring
  instead of st_ps (kernel_v5/v6/v8 baks) — wk tenants are drained by ACT
  copies that queue behind the kt-loop exp backlog, stalling the next
  pair's decompress worse than the aux-buffer stall it removed (+26us both
  times measured). Note the device clock state adds
  ~±4% run-to-run (HAM/P0 downclock under sustained load drifts
  per-matmul time 273->321ns), so only deltas beyond that are trusted.
"""

import os
import sys

for _p in ("/root/.axon_site/_ro/trn_rl_repo", "/opt/trn_rl_repo"):
    if os.path.isdir(_p) and _p not in sys.path:
        sys.path.insert(0, _p)

import numpy as np

import concourse.bass as bass
import concourse.tile as tile
from concourse import mybir
from concourse.bass_utils import run_bass_kernel_spmd

F32 = mybir.dt.float32
F32R = mybir.dt.float32r
BF16 = mybir.dt.bfloat16
F8 = mybir.dt.float8e4
DR = mybir.MatmulPerfMode.DoubleRow

D = 2048        # d_model
S = 2048        # seq len
B = 2           # batch
H = 16          # heads
HD = 128        # nope head dim
KV = 512        # kv lora rank
QL = 768        # q lora rank
RD = 64         # rope dim
EPS = 1e-6
SQ = 512        # query rows per core
N_CORES = 8
GROUPS = 4      # head groups of 4
GH = 4          # heads per group
SCALE = 1.0 / float(np.sqrt(HD + RD))

NKV = KV // 128   # 4 kv-lora chunks
NQL = QL // 128   # 6 q-lora chunks
NS = S // 512     # 4 seq blocks
NST = S // 128    # 16 seq tiles


# ---------------------------------------------------------------------------
# The walrus build in this container only encodes a single sync-wait on a
# Drain (TPB_CTRL) instruction, but TileContext._drain_and_barrier parks the
# whole global-clock wait set on the tail drain ("Too many sync wait
# commands"). Hoist the waits onto single-wait NOPs ahead of a bare drain.
def _patch_tile_drain():
    from bass_rust import ScopedClock

    def _drain_and_barrier(self, tick_clock, wait_clock):
        probe = self.nc.sync.nop(nofuse=True)
        wait_clock.add_sem_waits(
            probe.ins, ScopedClock({None: tick_clock.global_clock})
        )
        si = probe.ins.sync_info
        waits = list(si.on_wait) if si is not None else []
        if len(waits) > 1:
            probe.ins.sync_info = mybir.SyncInfo(on_wait=waits[:1], on_update=[])
            for w in waits[1:]:
                extra = self.nc.sync.nop(nofuse=True)
                extra.ins.sync_info = mybir.SyncInfo(on_wait=[w], on_update=[])
        self.nc.sync.drain()

        self.nc.all_engine_barrier()
        assert self.sems is not None
        popped = self.nc._tile_sem_poison_stack.pop()
        assert popped is self._sem_poison
        self.nc.clear_and_free_semaphores(list(self.sems.allocated().values()))
        self.nc.all_engine_barrier()

    tile.TileContext._drain_and_barrier = _drain_and_barrier


_patch_tile_drain()


def _r(ap):
    return ap.bitcast(F32R)


def build_nc():
    nc = bass.Bass()

    xT = nc.dram_tensor("xT", [D, S], BF16, kind="ExternalInput")
    xqT = nc.dram_tensor("xqT", [D, SQ], BF16, kind="ExternalInput")
    wcq = nc.dram_tensor("wcq", [D, QL], BF16, kind="ExternalInput")
    wckv = nc.dram_tensor("wckv", [D, KV], BF16, kind="ExternalInput")
    wkr2 = nc.dram_tensor("wkr2", [D, 128], BF16, kind="ExternalInput")
    wdq = nc.dram_tensor("wdq", [QL, H * HD], BF16, kind="ExternalInput")
    wdqr = nc.dram_tensor("wdqr", [QL, H * RD], BF16, kind="ExternalInput")
    wdk = nc.dram_tensor("wdk", [KV, H * HD], BF16, kind="ExternalInput")
    wdv = nc.dram_tensor("wdv", [KV, H * HD], BF16, kind="ExternalInput")
    wo = nc.dram_tensor("wo", [H * HD, D], BF16, kind="ExternalInput")
    gq = nc.dram_tensor("gq", [1, QL], F32, kind="ExternalInput")
    gkv = nc.dram_tensor("gkv", [1, KV], F32, kind="ExternalInput")
    cosk = nc.dram_tensor("cosk", [128, S], F32, kind="ExternalInput")
    sink = nc.dram_tensor("sink", [128, S], F32, kind="ExternalInput")
    cosq = nc.dram_tensor("cosq", [128, SQ], F32, kind="ExternalInput")
    sinq = nc.dram_tensor("sinq", [128, SQ], F32, kind="ExternalInput")
    rotp = nc.dram_tensor("rotp", [128, 128], BF16, kind="ExternalInput")
    out = nc.dram_tensor("out", [SQ, D], F32, kind="ExternalOutput")
    debug = bool(int(os.environ.get("MLA_DEBUG", "0")))
    if debug:
        dbg_kvcT = nc.dram_tensor("dbg_kvcT", [KV, S], BF16, kind="ExternalOutput")
        dbg_qcT = nc.dram_tensor("dbg_qcT", [QL, SQ], BF16, kind="ExternalOutput")
        dbg_krT = nc.dram_tensor("dbg_krT", [128, S], BF16, kind="ExternalOutput")
        dbg_oT = nc.dram_tensor("dbg_oT", [H * 128, SQ], BF16, kind="ExternalOutput")

    with tile.TileContext(nc) as tc:
        _build_body(nc, tc, locals(), debug)
    _split_excess_waits(nc)
    return nc


# This walrus build encodes at most one sync-wait per engine instruction;
# hoist surplus waits onto single-wait NOPs right before the instruction on
# the same engine queue (in-order execution keeps the semantics identical).
def _split_excess_waits(nc, max_waits=1):
    n_nops = 0
    for f in nc.m.functions:
        for bb in f.blocks:
            out = []
            for ins in bb.instructions:
                si = ins.sync_info
                if si is not None:
                    sem = [w for w in si.on_wait if w.sync_type == "semaphore"]
                    other = [w for w in si.on_wait if w.sync_type != "semaphore"]
                    budget = max(max_waits - len(other), 0)
                    if len(sem) > budget:
                        extra, keep = sem[:-budget] if budget else sem, (
                            sem[-budget:] if budget else [])
                        for j, w in enumerate(extra):
                            nop = mybir.InstNoOp(
                                name=f"{ins.name}-wsplit{j}",
                                engine=ins.engine,
                                bass_nofuse=True,
                                sync_info=mybir.SyncInfo(
                                    on_wait=[w], on_update=[]),
                            )
                            out.append(nop)
                            n_nops += 1
                        ins.sync_info = mybir.SyncInfo(
                            on_wait=other + keep,
                            on_update=list(si.on_update))
                out.append(ins)
            bb.instructions = out
    return n_nops


def _norm_den(nc, misc, st_ps, ones128, pending):
    """Stage 1 of softmax normalization: den1 = sum_partitions(den) via a
    ones-matmul into row 0 of an st_ps tile. The st ring's previous tenants
    are score tiles already drained by exp, so unlike the aux ring (whose
    tenant is the rope rot tile with a slow DVE consumer) or the wk ring
    (whose tenants gate the next pair's decompress), nothing stalls."""
    recs = []
    for h, pot, den in pending:
        dst = st_ps.tile([128, SQ], F32, tag="st", name="dst")
        nc.tensor.matmul(dst[0:1, :], _r(ones128), _r(den))
        rec = misc.tile([1, SQ], F32R, tag="rec")
        nc.scalar.copy(rec, dst[0:1, :])
        with nc.allow_low_precision(reason="f32r is full fp32 bits"):
            nc.vector.reciprocal(rec, rec)
        recs.append(rec)
    return recs


def _norm_fin(nc, misc, st_ps, oT, ones1, pending, recs):
    """Stage 2: broadcast 1/den across partitions and scale the attention
    output into oT[h]. Emitted after the rope matmuls so the DVE reciprocal
    latency is hidden behind PE work."""
    for (h, pot, den), rec in zip(pending, recs):
        bc = st_ps.tile([128, SQ], F32, tag="st", name="bcst")
        nc.tensor.matmul(bc, _r(ones1), _r(rec))
        bc_s = misc.tile([128, SQ], F32, tag="bcs")
        nc.scalar.copy(bc_s, bc)
        nc.vector.tensor_mul(oT[h], pot, bc_s)


def _build_body(nc, tc, t, debug=False):
    from contextlib import ExitStack

    ctx = ExitStack()
    with ctx:
        consts = ctx.enter_context(tc.tile_pool(name="consts", bufs=1))
        persist = ctx.enter_context(tc.tile_pool(name="persist", bufs=1))
        misc = ctx.enter_context(tc.tile_pool(name="misc", bufs=2))
        # PSUM pools: aux first (lives through whole kernel), then phase pools.
        aux_ps = ctx.enter_context(tc.tile_pool(name="aux_ps", bufs=1, space="PSUM"))

        # ---- constants -----------------------------------------------------
        ones128f = consts.tile([128, 1], F32)
        nc.vector.memset(ones128f, 1.0)
        ones128 = consts.tile([128, 1], F32R)
        nc.scalar.copy(ones128, ones128f)
        ones1f = consts.tile([1, 128], F32)
        nc.vector.memset(ones1f, 1.0)
        ones1 = consts.tile([1, 128], F32R)
        nc.scalar.copy(ones1, ones1f)
        gq_s = consts.tile([1, QL], F32R)
        nc.sync.dma_start(out=gq_s, in_=t["gq"][:, :].bitcast(F32R))
        gkv_s = consts.tile([1, KV], F32R)
        nc.sync.dma_start(out=gkv_s, in_=t["gkv"][:, :].bitcast(F32R))
        eps_s = consts.tile([1, 1], F32)
        nc.vector.memset(eps_s, EPS)
        rotp_s = consts.tile([128, 128], BF16)
        nc.sync.dma_start(out=rotp_s, in_=t["rotp"][:, :])
        cosq_s = consts.tile([128, SQ], F32)
        nc.sync.dma_start(out=cosq_s, in_=t["cosq"][:, :])
        sinq_s = consts.tile([128, SQ], F32)
        nc.sync.dma_start(out=sinq_s, in_=t["sinq"][:, :])

        # ---- persistent tiles (bf16: matmul operands -> FWL weight loads) --
        kvcT = [persist.tile([128, S], BF16, tag=f"kvcT{c}", name=f"kvcT{c}") for c in range(NKV)]
        krT = persist.tile([128, S], BF16, tag="krT")
        qcT = [persist.tile([128, SQ], BF16, tag=f"qcT{c}", name=f"qcT{c}") for c in range(NQL)]
        oT = [persist.tile([128, SQ], BF16, tag=f"oT{h}", name=f"oT{h}") for h in range(H)]

        # ===================================================================
        # Phase 0: compress. kvcT/krT over full seq, qcT over own query rows.
        # ===================================================================
        with nc.named_scope("p0_compress", notify=True), \
             tc.tile_pool(name="misc0", bufs=2) as misc0, \
             tc.tile_pool(name="xtp", bufs=3) as xtp, \
             tc.tile_pool(name="wkvhold", bufs=1) as wkvhold, \
             tc.tile_pool(name="wstream", bufs=3) as wstream, \
             tc.tile_pool(name="acc_ps", bufs=6, space="PSUM") as acc_ps:
            # wckv/wkr are reused by all 4 seq blocks: load once, keep in SBUF
            wkv_h = [wkvhold.tile([128, KV], BF16, tag=f"wckv{d}", name=f"wckv{d}")
                     for d in range(16)]
            wkr_h = [wkvhold.tile([128, 128], BF16, tag=f"wkr{d}", name=f"wkr{d}")
                     for d in range(16)]
            def p0_post(scol, kvraw, kraw):
                # rmsnorm over kv features (partition dim across the 4 chunks)
                ssq = aux_ps.tile([1, 512], F32, tag="aux")
                for c in range(NKV):
                    sq = misc0.tile([128, 512], F32R, tag="sq")
                    nc.scalar.square(sq, kvraw[c])
                    nc.tensor.matmul(ssq, _r(ones128), _r(sq),
                                     start=(c == 0), stop=(c == NKV - 1))
                rstd = misc0.tile([1, 512], F32R, tag="rstd")
                nc.scalar.activation(rstd, ssq,
                                     mybir.ActivationFunctionType.Sqrt,
                                     bias=eps_s[:, :], scale=1.0 / KV)
                with nc.allow_low_precision(reason="f32r is full fp32 bits"):
                    nc.vector.reciprocal(rstd, rstd)
                for c in range(NKV):
                    bc = aux_ps.tile([128, 512], F32, tag="aux")
                    nc.tensor.matmul(
                        bc, _r(gkv_s[:, c * 128:(c + 1) * 128]), _r(rstd))
                    bc_s = misc.tile([128, 512], F32, tag="bcs")
                    nc.scalar.copy(bc_s, bc)
                    nc.vector.tensor_mul(kvcT[c][:, scol], kvraw[c], bc_s)

                # rope on the (duplicated-rows) k_rope block
                ck = misc0.tile([128, 512], F32, tag="ck")
                nc.sync.dma_start(out=ck, in_=t["cosk"][:, scol])
                sk = misc0.tile([128, 512], F32, tag="sk")
                nc.sync.dma_start(out=sk, in_=t["sink"][:, scol])
                rot = aux_ps.tile([128, 512], F32, tag="aux")
                nc.tensor.matmul(rot, rotp_s, kraw)
                t1 = misc0.tile([128, 512], F32, tag="ropet1")
                nc.vector.tensor_mul(t1, kraw, ck)
                t2 = misc0.tile([128, 512], F32, tag="ropet2")
                nc.vector.tensor_mul(t2, rot, sk)
                nc.vector.tensor_add(krT[:, scol], t1, t2)

            p0_pending = None
            for sb in range(NS):
                scol = slice(sb * 512, (sb + 1) * 512)
                pkv = [acc_ps.tile([128, 512], F32, tag="acc", name="pkv") for _ in range(NKV)]
                pkr = acc_ps.tile([128, 512], F32, tag="acc")
                for d in range(16):
                    drow = slice(d * 128, (d + 1) * 128)
                    if sb == 0:
                        nc.sync.dma_start(out=wkv_h[d], in_=t["wckv"][drow, :])
                        nc.sync.dma_start(out=wkr_h[d], in_=t["wkr2"][drow, :])
                    xt = xtp.tile([128, 512], BF16, tag="xt")
                    nc.sync.dma_start(out=xt, in_=t["xT"][drow, scol])
                    for c in range(NKV):
                        nc.tensor.matmul(
                            pkv[c], wkv_h[d][:, c * 128:(c + 1) * 128], xt,
                            start=(d == 0), stop=(d == 15))
                    nc.tensor.matmul(pkr, wkr_h[d], xt,
                                     start=(d == 0), stop=(d == 15))
                # drain psum to raw bf16 sbuf tiles (releases acc banks), then
                # run the PREVIOUS block's normalize behind this block's MMs.
                kvraw = [misc0.tile([128, 512], BF16, tag=f"kvraw{c}",
                                    name=f"kvraw{c}")
                         for c in range(NKV)]
                for c in range(NKV):
                    nc.scalar.copy(kvraw[c], pkv[c])
                kraw = misc0.tile([128, 512], BF16, tag="kraw")
                nc.scalar.copy(kraw, pkr)
                if p0_pending is not None:
                    p0_post(*p0_pending)
                p0_pending = (scol, kvraw, kraw)

            # qcT over own query rows
            pqc = [acc_ps.tile([128, 512], F32, tag="acc", name="pqc") for _ in range(NQL)]
            for d in range(16):
                drow = slice(d * 128, (d + 1) * 128)
                xt = xtp.tile([128, 512], BF16, tag="xt")
                nc.sync.dma_start(out=xt, in_=t["xqT"][drow, :])
                wq_t = wstream.tile([128, QL], BF16, tag="wcq")
                nc.sync.dma_start(out=wq_t, in_=t["wcq"][drow, :])
                for c in range(NQL):
                    nc.tensor.matmul(
                        pqc[c], wq_t[:, c * 128:(c + 1) * 128], xt,
                        start=(d == 0), stop=(d == 15))
            if p0_pending is not None:
                p0_post(*p0_pending)
                p0_pending = None
            ssq = aux_ps.tile([1, 512], F32, tag="aux")
            for c in range(NQL):
                sq = misc0.tile([128, 512], F32R, tag="sq")
                nc.scalar.square(sq, pqc[c])
                nc.tensor.matmul(ssq, _r(ones128), _r(sq),
                                 start=(c == 0), stop=(c == NQL - 1))
            rstd = misc0.tile([1, 512], F32R, tag="rstd")
            nc.scalar.activation(rstd, ssq, mybir.ActivationFunctionType.Sqrt,
                                 bias=eps_s[:, :], scale=1.0 / QL)
            with nc.allow_low_precision(reason="f32r is full fp32 bits"):
                nc.vector.reciprocal(rstd, rstd)
            for c in range(NQL):
                bc = aux_ps.tile([128, 512], F32, tag="aux")
                nc.tensor.matmul(bc, _r(gq_s[:, c * 128:(c + 1) * 128]), _r(rstd))
                bc_s = misc.tile([128, 512], F32, tag="bcs")
                nc.scalar.copy(bc_s, bc)
                nc.vector.tensor_mul(qcT[c], pqc[c], bc_s)

        # ===================================================================
        # Phase A: per head group -- decompress k/v/q, attention.
        # Scores run as fp8e4 DoubleRow matmuls: contraction slots [p, i]
        # hold nope dims (i=0) and rope dims (i=1, rows 0:64 for even heads /
        # 64:128 for odd heads, zero elsewhere), so one K=256 DR matmul per
        # (head, kt) replaces the K=128 nope + K=64 rope pair. fp8 on the
        # score operands costs ~0.9% rel err on the output (logit errors are
        # shrunk by SCALE before exp; measured in quant_study.py).
        # ===================================================================
        with nc.named_scope("pA_attn", notify=True), \
             tc.tile_pool(name="vpool", bufs=24) as vpool, \
             tc.tile_pool(name="khp", bufs=2) as khp, \
             tc.tile_pool(name="qmp", bufs=2) as qmp, \
             tc.tile_pool(name="ptp", bufs=4) as ptp, \
             tc.tile_pool(name="denp", bufs=2) as denp, \
             tc.tile_pool(name="wdqp", bufs=6) as wdqp, \
             tc.tile_pool(name="wdqrp", bufs=6) as wdqrp, \
             tc.tile_pool(name="wdkp", bufs=4) as wdkp, \
             tc.tile_pool(name="wdvp", bufs=4) as wdvp, \
             tc.tile_pool(name="st_ps", bufs=3, space="PSUM") as st_ps, \
             tc.tile_pool(name="ot_ps", bufs=2, space="PSUM") as ot_ps, \
             tc.tile_pool(name="wk_ps", bufs=2, space="PSUM") as wk_ps:

            # kr-with-zeros fp8 patterns DMAd into each pair's kh[:, 1, :]:
            # krzA rows 0:64 = kr (even head), krzB rows 64:128 = kr (odd).
            krzA = persist.tile([128, S], F8, tag="krzA")
            krzB = persist.tile([128, S], F8, tag="krzB")
            nc.vector.memset(krzA, 0.0)
            nc.vector.memset(krzB, 0.0)
            nc.scalar.copy(krzA[0:64, :], krT[0:64, :])
            nc.scalar.copy(krzB[64:128, :], krT[64:128, :])

            pending = []
            for g in range(GROUPS):
                gcol = slice(g * 512, (g + 1) * 512)
                # stream this group's decompress weights
                wdv_t = [wdvp.tile([128, 512], BF16, tag="wdv", name="wdv_t") for _ in range(NKV)]
                for c in range(NKV):
                    nc.sync.dma_start(
                        out=wdv_t[c], in_=t["wdv"][c * 128:(c + 1) * 128, gcol])
                wdk_t = [wdkp.tile([128, 512], BF16, tag="wdk", name="wdk_t") for _ in range(NKV)]
                for c in range(NKV):
                    nc.sync.dma_start(
                        out=wdk_t[c], in_=t["wdk"][c * 128:(c + 1) * 128, gcol])
                wdq_t = [wdqp.tile([128, 512], BF16, tag="wdq", name="wdq_t") for _ in range(NQL)]
                for c in range(NQL):
                    nc.sync.dma_start(
                        out=wdq_t[c], in_=t["wdq"][c * 128:(c + 1) * 128, gcol])
                grcol = slice(g * 256, (g + 1) * 256)
                wdqr_t = [wdqrp.tile([128, 256], BF16, tag="wdqr", name="wdqr_t") for _ in range(NQL)]
                for c in range(NQL):
                    nc.sync.dma_start(
                        out=wdqr_t[c], in_=t["wdqr"][c * 128:(c + 1) * 128, grcol])

                # v for all 4 heads of the group: moving = wdv (512 wide),
                # stationary = kvc seq-tile. Halves the matmul/LDW count vs
                # the per-pair 256-wide variant.
                vt = {}
                for st in range(NST):
                    pv = wk_ps.tile([128, 512], F32, tag="wk")
                    for c in range(NKV):
                        nc.tensor.matmul(
                            pv, kvcT[c][:, st * 128:(st + 1) * 128], wdv_t[c],
                            start=(c == 0), stop=(c == NKV - 1))
                    v_s = vpool.tile([128, 512], BF16, tag="v")
                    nc.scalar.copy(v_s, pv)
                    vt[st] = v_s

                for pair in range(GH // 2):
                    hA = g * GH + 2 * pair
                    hB = hA + 1
                    colA = slice((2 * pair) * 128, (2 * pair + 1) * 128)
                    colB = slice((2 * pair + 1) * 128, (2 * pair + 2) * 128)

                    # k^T DoubleRow tiles for both heads: [128, 2, S] fp8
                    # (i=0 nope from decompress, i=1 rope pattern via DMA)
                    khA = khp.tile([128, 2, S], F8, tag="kh")
                    khB = khp.tile([128, 2, S], F8, tag="kh")
                    nc.sync.dma_start(out=khA[:, 1, :], in_=krzA[:, :])
                    nc.sync.dma_start(out=khB[:, 1, :], in_=krzB[:, :])
                    for kh, hcol in ((khA, colA), (khB, colB)):
                        for blk in range(NS):
                            bcol = slice(blk * 512, (blk + 1) * 512)
                            pk = wk_ps.tile([128, 512], F32, tag="wk")
                            for c in range(NKV):
                                nc.tensor.matmul(
                                    pk, wdk_t[c][:, hcol], kvcT[c][:, bcol],
                                    start=(c == 0), stop=(c == NKV - 1))
                            nc.scalar.copy(kh[:, 0, bcol], pk)

                    # q DoubleRow tiles for both heads: [128, 2, SQ] fp8
                    # (i=0 nope, i=1 roped q_rope -- both heads' rope rows,
                    # masked by the zeros in the kh rope pattern)
                    qmA = qmp.tile([128, 2, SQ], F8, tag="qm")
                    qmB = qmp.tile([128, 2, SQ], F8, tag="qm")
                    for qm, hcol in ((qmA, colA), (qmB, colB)):
                        pq = wk_ps.tile([128, SQ], F32, tag="wk")
                        for c in range(NQL):
                            nc.tensor.matmul(pq, wdq_t[c][:, hcol], qcT[c],
                                             start=(c == 0), stop=(c == NQL - 1))
                        nc.scalar.copy(qm[:, 0, :], pq)

                    # normalize the PREVIOUS pair, stage 1 (its den DVE
                    # chain drained behind the decompress matmuls above)
                    recs = _norm_den(nc, misc, st_ps, ones128, pending)

                    # q_rope for the pair (two heads stacked on partitions)
                    prcol = slice(pair * 128, (pair + 1) * 128)
                    pqr = wk_ps.tile([128, SQ], F32, tag="wk")
                    for c in range(NQL):
                        nc.tensor.matmul(
                            pqr, wdqr_t[c][:, prcol], qcT[c],
                            start=(c == 0), stop=(c == NQL - 1))
                    qraw = misc.tile([128, SQ], BF16, tag="qraw")
                    nc.scalar.copy(qraw, pqr)
                    rot = aux_ps.tile([128, SQ], F32, tag="aux")
                    nc.tensor.matmul(rot, rotp_s, qraw)
                    t1 = misc.tile([128, SQ], F32, tag="ropet1")
                    nc.vector.tensor_mul(t1, qraw, cosq_s)
                    t2 = misc.tile([128, SQ], F32, tag="ropet2")
                    nc.vector.tensor_mul(t2, rot, sinq_s)
                    nc.vector.tensor_add(qmA[:, 1, :], t1, t2)
                    nc.vector.tensor_add(qmB[:, 1, :], t1, t2)

                    # ...stage 2 lands after the rope matmuls so the DVE
                    # reciprocal latency is PE-covered.
                    _norm_fin(nc, misc, st_ps, oT, ones1, pending, recs)
                    pending.clear()

                    # attention for the pair: one fp8 DoubleRow matmul per
                    # (head, kt) covers nope + rope contraction.
                    potA = ot_ps.tile([128, SQ], F32, tag="ot")
                    potB = ot_ps.tile([128, SQ], F32, tag="ot")
                    denA = denp.tile([128, SQ], F32R, tag="den")
                    denB = denp.tile([128, SQ], F32R, tag="den")
                    for kt in range(NST):
                        kcol = slice(kt * 128, (kt + 1) * 128)
                        pstA = st_ps.tile([128, SQ], F32, tag="st")
                        pstB = st_ps.tile([128, SQ], F32, tag="st")
                        nc.tensor.matmul(pstA, khA[:, :, kcol], qmA[:, :, :],
                                         perf_mode=DR, start=True, stop=True)
                        nc.tensor.matmul(pstB, khB[:, :, kcol], qmB[:, :, :],
                                         perf_mode=DR, start=True, stop=True)
                        ptA = ptp.tile([128, SQ], BF16, tag="pt")
                        nc.scalar.activation(ptA, pstA,
                                             mybir.ActivationFunctionType.Exp,
                                             scale=SCALE)
                        ptB = ptp.tile([128, SQ], BF16, tag="pt")
                        nc.scalar.activation(ptB, pstB,
                                             mybir.ActivationFunctionType.Exp,
                                             scale=SCALE)
                        if kt == 0:
                            nc.vector.tensor_copy(denA, ptA)
                            nc.vector.tensor_copy(denB, ptB)
                        else:
                            nc.vector.tensor_add(denA, denA, ptA)
                            nc.vector.tensor_add(denB, denB, ptB)
                        vs = vt[kt]
                        nc.tensor.matmul(
                            potA, vs[:, colA], ptA,
                            start=(kt == 0), stop=(kt == NST - 1))
                        nc.tensor.matmul(
                            potB, vs[:, colB], ptB,
                            start=(kt == 0), stop=(kt == NST - 1))

                    pending.append((hA, potA, denA))
                    pending.append((hB, potB, denB))

            recs = _norm_den(nc, misc, st_ps, ones128, pending)
            _norm_fin(nc, misc, st_ps, oT, ones1, pending, recs)
            pending.clear()

        if debug:
            for c in range(NKV):
                nc.sync.dma_start(
                    out=t["dbg_kvcT"][c * 128:(c + 1) * 128, :], in_=kvcT[c])
            for c in range(NQL):
                nc.sync.dma_start(
                    out=t["dbg_qcT"][c * 128:(c + 1) * 128, :], in_=qcT[c])
            nc.sync.dma_start(out=t["dbg_krT"][:, :], in_=krT)
            for h in range(H):
                nc.sync.dma_start(
                    out=t["dbg_oT"][h * 128:(h + 1) * 128, :], in_=oT[h])

        # ===================================================================
        # Phase B: output projection, all 16 heads, PSUM-accumulated.
        # Loop order: wo tile loads once per (h, blk) and serves all 4 query
        # tiles (wo HBM traffic 16MB instead of 64MB).
        # ===================================================================
        NQT = SQ // 128
        with nc.named_scope("pB_outproj", notify=True), \
             tc.tile_pool(name="wop", bufs=8) as wop, \
             tc.tile_pool(name="outs", bufs=4) as outs, \
             tc.tile_pool(name="po_ps", bufs=4, space="PSUM") as po_ps:
            for blk in range(NS):
                bcol = slice(blk * 512, (blk + 1) * 512)
                po = [po_ps.tile([128, 512], F32, tag="po", name=f"po{qt}")
                      for qt in range(NQT)]
                for h in range(H):
                    wo_t = wop.tile([128, 512], BF16, tag="wo")
                    nc.sync.dma_start(
                        out=wo_t, in_=t["wo"][h * 128:(h + 1) * 128, bcol])
                    for qt in range(NQT):
                        nc.tensor.matmul(
                            po[qt], oT[h][:, qt * 128:(qt + 1) * 128], wo_t,
                            start=(h == 0), stop=(h == H - 1))
                for qt in range(NQT):
                    o_s = outs.tile([128, 512], F32, tag="os")
                    nc.scalar.copy(o_s, po[qt])
                    nc.sync.dma_start(
                        out=t["out"][qt * 128:(qt + 1) * 128, bcol], in_=o_s)


_NC_CACHE = None


def _get_nc():
    global _NC_CACHE
    if _NC_CACHE is None:
        _NC_CACHE = build_nc()
    return _NC_CACHE


def _rope_tables(positions):
    """cos/sin tables in transposed-packed layout [128, len(positions)]:
    rows 0:64 and 64:128 both hold the [RD, s] table (two rope vectors are
    stacked per 128 partitions)."""
    inv_freq = 1.0 / (10000.0 ** (np.arange(0, RD, 2, dtype=np.float32) / RD))
    ang = positions[:, None].astype(np.float32) * inv_freq[None, :]  # [s, 32]
    cos = np.concatenate([np.cos(ang), np.cos(ang)], axis=-1)        # [s, 64]
    sin = np.concatenate([np.sin(ang), np.sin(ang)], axis=-1)
    cosT = np.ascontiguousarray(cos.T)                               # [64, s]
    sinT = np.ascontiguousarray(sin.T)
    return (np.concatenate([cosT, cosT], axis=0),
            np.concatenate([sinT, sinT], axis=0))


def _rot_perm():
    m = np.zeros((128, 128), dtype=np.float32)
    for b0 in (0, 64):
        for i in range(32):
            m[b0 + i + 32, b0 + i] = -1.0   # rot[m] = -t[m+32], m < 32
            m[b0 + i, b0 + i + 32] = 1.0    # rot[m] = +t[m-32], m >= 32
    return m


def kernel(x, Wcq, g_q, Wdq, Wdqr, Wckv, g_kv, Wdk, Wdv, Wkr, Wo):
    import ml_dtypes

    bf16 = ml_dtypes.bfloat16
    nc = _get_nc()

    x = np.asarray(x, dtype=np.float32)
    xT = [np.ascontiguousarray(x[b].T).astype(bf16) for b in range(B)]  # [D, S]
    wkr2 = np.ascontiguousarray(
        np.concatenate([Wkr, Wkr], axis=1)).astype(bf16)  # [D, 128]
    cosk, sink = _rope_tables(np.arange(S))
    rotp = _rot_perm().astype(bf16)

    shared = {
        "wcq": np.ascontiguousarray(Wcq).astype(bf16),
        "wckv": np.ascontiguousarray(Wckv).astype(bf16),
        "wkr2": wkr2,
        "wdq": np.ascontiguousarray(Wdq).astype(bf16),
        "wdqr": np.ascontiguousarray(Wdqr).astype(bf16),
        "wdk": np.ascontiguousarray(Wdk).astype(bf16),
        "wdv": np.ascontiguousarray(Wdv).astype(bf16),
        "wo": np.ascontiguousarray(Wo).astype(bf16),
        "gq": np.ascontiguousarray(g_q, dtype=np.float32).reshape(1, QL),
        "gkv": np.ascontiguousarray(g_kv, dtype=np.float32).reshape(1, KV),
        "cosk": np.ascontiguousarray(cosk),
        "sink": np.ascontiguousarray(sink),
        "rotp": rotp,
    }

    in_maps = []
    for core in range(N_CORES):
        b, sl = core // 4, core % 4
        rows = np.arange(sl * SQ, (sl + 1) * SQ)
        cq, sq_t = _rope_tables(rows)
        m = dict(shared)
        m["xT"] = xT[b]
        m["xqT"] = np.ascontiguousarray(xT[b][:, sl * SQ:(sl + 1) * SQ])
        m["cosq"] = np.ascontiguousarray(cq)
        m["sinq"] = np.ascontiguousarray(sq_t)
        in_maps.append(m)

    trace = bool(int(os.environ.get("MLA_TRACE", "0")))
    res = run_bass_kernel_spmd(
        nc, in_maps, core_ids=list(range(N_CORES)), trace=trace,
        trace_cores=list(range(N_CORES)) if trace else None,
        stitch_traces=bool(int(os.environ.get("MLA_STITCH", "0"))),
        tmpdir=os.environ.get("MLA_TMPDIR") or None,
    )
    kernel.last_result = res

    out = np.empty((B, S, D), dtype=np.float32)
    for core in range(N_CORES):
        b, sl = core // 4, core % 4
        out[b, sl * SQ:(sl + 1) * SQ, :] = res.results[core]["out"]
    return out



# revision 15
# speedup vs baseline: 1.4891x; 1.1925x over previous
"""MLA forward Bass kernel for 8 TRN2 NeuronCores.

Sharding: pure query-row sharding. Core c handles batch b = c//4 and query rows
[sl*512, (sl+1)*512) with sl = c%4, for ALL 16 heads. Keys/values span the full
sequence, so the compressed-KV path (kvc, k_rope) is computed per-core for the
whole batch (replicated across the 4 cores that share a batch), while the Q
path, attention, and the output projection only cover the core's 512 query
rows. The full output-projection contraction (all 16 heads) is local, so no
cross-core reduction is needed: the host just concatenates the 8 row-blocks.

Layouts: everything TensorE-facing is kept transposed ([feature, seq]) so the
feature dim sits on partitions and matmuls contract over it. Softmax runs on
S^T tiles [k, q]: exp on ACT (no max-shift; scores are O(1) here), denominator
via DVE tile-adds + a ones-matmul partition reduction, normalization folded
into the PSUM->SBUF drain of the attention output. RoPE's rotate-half is a
constant 128x128 permutation matmul.

Perf notes (measured on HW via NTFF traces; baseline 1,197,956 ns):
- All big matmul operands are bf16: same PE row rate as f32r at free>=256,
  but LDWEIGHTS runs at 1 cyc/row with compiler-auto FWL (f32 is 2 cyc/row
  and FWL-ineligible) and weight/x DMA bytes halve. PSUM accum stays fp32;
  rel_fro vs the fp32 reference is ~5e-3 (budget 2e-2).
- Attention processes head PAIRS with softmax normalization deferred by one
  pair and split in two stages: the den reductions (_norm_den) queue behind
  the next pair's decompress matmuls, the broadcast+scale (_norm_fin) lands
  after the rope matmuls so the DVE reciprocal latency is PE-covered. The
  den1/bc tiles live in the st_ps ring, whose previous tenants (score tiles)
  are already drained by exp — the aux ring's tenant (rope rot, slow DVE
  consumer) stalled ~2.4us per pair, and a wk_ps variant stalled the next
  pair's decompress (see below).
- Phase 0 drains compress PSUM to raw bf16 tiles and defers each block's
  rmsnorm/rope postprocessing behind the next block's matmuls.
- wo streams once per (head, block) serving all 4 query tiles; wckv/wkr are
  held in SBUF across seq blocks; prologue weight DMAs interleave with the
  first block's x tiles (first matmul ~14us in instead of ~36us); each
  group's wdv tiles fetch before wdk (v is consumed first); wo prefetch is
  8 tiles deep.
- Tried and reverted: seq-sharding the kv compress via a 4-core DRAM
  AllGather (kernel_v3_ag.py.bak) — the collective costs ~90us on this
  axon path, washing out the ~58us of saved PE rows; aux_ps=2/st_ps=2
  pool sizing; and routing the normalize split through the wk_ps ring
  instead of st_ps (kernel_v5/v6/v8 baks) — wk tenants are drained by ACT
  copies that queue behind the kt-loop exp backlog, stalling the next
  pair's decompress worse than the aux-buffer stall it removed (+26us both
  times measured). Note the device clock state adds
  ~±4% run-to-run (HAM/P0 downclock under sustained load drifts
  per-matmul time 273->321ns), so only deltas beyond that are trusted.
"""

import os
import sys

for _p in ("/root/.axon_site/_ro/trn_rl_repo", "/opt/trn_rl_repo"):
    if os.path.isdir(_p) and _p not in sys.path:
        sys.path.insert(0, _p)

import numpy as np

import concourse.bass as bass
import concourse.tile as tile
from concourse import mybir
from concourse.bass_utils import run_bass_kernel_spmd

F32 = mybir.dt.float32
F32R = mybir.dt.float32r
BF16 = mybir.dt.bfloat16
F8 = mybir.dt.float8e4
DR = mybir.MatmulPerfMode.DoubleRow

D = 2048        # d_model
S = 2048        # seq len
B = 2           # batch
H = 16          # heads
HD = 128        # nope head dim
KV = 512        # kv lora rank
QL = 768        # q lora rank
RD = 64         # rope dim
EPS = 1e-6
SQ = 512        # query rows per core
N_CORES = 8
GROUPS = 4      # head groups of 4
GH = 4          # heads per group
SCALE = 1.0 / float(np.sqrt(HD + RD))

NKV = KV // 128   # 4 kv-lora chunks
NQL = QL // 128   # 6 q-lora chunks
NS = S // 512     # 4 seq blocks
NST = S // 128    # 16 seq tiles


# ---------------------------------------------------------------------------
# The walrus build in this container only encodes a single sync-wait on a
# Drain (TPB_CTRL) instruction, but TileContext._drain_and_barrier parks the
# whole global-clock wait set on the tail drain ("Too many sync wait
# commands"). Hoist the waits onto single-wait NOPs ahead of a bare drain.
def _patch_tile_drain():
    from bass_rust import ScopedClock

    def _drain_and_barrier(self, tick_clock, wait_clock):
        probe = self.nc.sync.nop(nofuse=True)
        wait_clock.add_sem_waits(
            probe.ins, ScopedClock({None: tick_clock.global_clock})
        )
        si = probe.ins.sync_info
        waits = list(si.on_wait) if si is not None else []
        if len(waits) > 1:
            probe.ins.sync_info = mybir.SyncInfo(on_wait=waits[:1], on_update=[])
            for w in waits[1:]:
                extra = self.nc.sync.nop(nofuse=True)
                extra.ins.sync_info = mybir.SyncInfo(on_wait=[w], on_update=[])
        self.nc.sync.drain()

        self.nc.all_engine_barrier()
        assert self.sems is not None
        popped = self.nc._tile_sem_poison_stack.pop()
        assert popped is self._sem_poison
        self.nc.clear_and_free_semaphores(list(self.sems.allocated().values()))
        self.nc.all_engine_barrier()

    tile.TileContext._drain_and_barrier = _drain_and_barrier


_patch_tile_drain()


def _r(ap):
    return ap.bitcast(F32R)


def build_nc():
    nc = bass.Bass()

    xT = nc.dram_tensor("xT", [D, S], BF16, kind="ExternalInput")
    wcq = nc.dram_tensor("wcq", [D, QL], BF16, kind="ExternalInput")
    wckv = nc.dram_tensor("wckv", [D, KV], BF16, kind="ExternalInput")
    wkr2 = nc.dram_tensor("wkr2", [D, 128], BF16, kind="ExternalInput")
    wdq = nc.dram_tensor("wdq", [QL, H * HD], BF16, kind="ExternalInput")
    wdqr = nc.dram_tensor("wdqr", [QL, H * RD], BF16, kind="ExternalInput")
    wdk = nc.dram_tensor("wdk", [KV, H * HD], BF16, kind="ExternalInput")
    wdv = nc.dram_tensor("wdv", [KV, H * HD], BF16, kind="ExternalInput")
    wo = nc.dram_tensor("wo", [H * HD, D], BF16, kind="ExternalInput")
    gq = nc.dram_tensor("gq", [1, QL], F32, kind="ExternalInput")
    gkv = nc.dram_tensor("gkv", [1, KV], F32, kind="ExternalInput")
    cosk = nc.dram_tensor("cosk", [128, S], F32, kind="ExternalInput")
    sink = nc.dram_tensor("sink", [128, S], F32, kind="ExternalInput")
    rotp = nc.dram_tensor("rotp", [128, 128], BF16, kind="ExternalInput")
    out = nc.dram_tensor("out", [SQ, D], F32, kind="ExternalOutput")
    debug = bool(int(os.environ.get("MLA_DEBUG", "0")))
    if debug:
        dbg_kvcT = nc.dram_tensor("dbg_kvcT", [KV, S], BF16, kind="ExternalOutput")
        dbg_qcT = nc.dram_tensor("dbg_qcT", [QL, SQ], BF16, kind="ExternalOutput")
        dbg_krT = nc.dram_tensor("dbg_krT", [128, S], BF16, kind="ExternalOutput")
        dbg_oT = nc.dram_tensor("dbg_oT", [H * 128, SQ], BF16, kind="ExternalOutput")

    with tile.TileContext(nc) as tc:
        _build_body(nc, tc, locals(), debug)
    _split_excess_waits(nc)
    return nc


# This walrus build encodes at most one sync-wait per engine instruction;
# hoist surplus waits onto single-wait NOPs right before the instruction on
# the same engine queue (in-order execution keeps the semantics identical).
def _split_excess_waits(nc, max_waits=1):
    n_nops = 0
    for f in nc.m.functions:
        for bb in f.blocks:
            out = []
            for ins in bb.instructions:
                si = ins.sync_info
                if si is not None:
                    sem = [w for w in si.on_wait if w.sync_type == "semaphore"]
                    other = [w for w in si.on_wait if w.sync_type != "semaphore"]
                    budget = max(max_waits - len(other), 0)
                    if len(sem) > budget:
                        extra, keep = sem[:-budget] if budget else sem, (
                            sem[-budget:] if budget else [])
                        for j, w in enumerate(extra):
                            nop = mybir.InstNoOp(
                                name=f"{ins.name}-wsplit{j}",
                                engine=ins.engine,
                                bass_nofuse=True,
                                sync_info=mybir.SyncInfo(
                                    on_wait=[w], on_update=[]),
                            )
                            out.append(nop)
                            n_nops += 1
                        ins.sync_info = mybir.SyncInfo(
                            on_wait=other + keep,
                            on_update=list(si.on_update))
                out.append(ins)
            bb.instructions = out
    return n_nops


def _norm_den(nc, misc, st_ps, ones128, pending):
    """Stage 1 of softmax normalization: den1 = sum_partitions(den) via a
    ones-matmul into row 0 of an st_ps tile. The st ring's previous tenants
    are score tiles already drained by exp, so unlike the aux ring (whose
    tenant is the rope rot tile with a slow DVE consumer) or the wk ring
    (whose tenants gate the next pair's decompress), nothing stalls."""
    recs = []
    for h, pot, den in pending:
        dst = st_ps.tile([128, SQ], F32, tag="st", name="dst")
        nc.tensor.matmul(dst[0:1, :], _r(ones128), _r(den))
        rec = misc.tile([1, SQ], F32R, tag="rec")
        nc.scalar.copy(rec, dst[0:1, :])
        with nc.allow_low_precision(reason="f32r is full fp32 bits"):
            nc.vector.reciprocal(rec, rec)
        recs.append(rec)
    return recs


def _norm_fin(nc, misc, st_ps, oT, ones1, pending, recs):
    """Stage 2: broadcast 1/den across partitions and scale the attention
    output into oT[h]. Emitted after the rope matmuls so the DVE reciprocal
    latency is hidden behind PE work."""
    for (h, pot, den), rec in zip(pending, recs):
        bc = st_ps.tile([128, SQ], F32, tag="st", name="bcst")
        nc.tensor.matmul(bc, _r(ones1), _r(rec))
        bc_s = misc.tile([128, SQ], F32, tag="bcs")
        nc.scalar.copy(bc_s, bc)
        nc.vector.tensor_mul(oT[h], pot, bc_s)


def _build_body(nc, tc, t, debug=False):
    from contextlib import ExitStack

    ctx = ExitStack()
    with ctx:
        consts = ctx.enter_context(tc.tile_pool(name="consts", bufs=1))
        persist = ctx.enter_context(tc.tile_pool(name="persist", bufs=1))
        misc = ctx.enter_context(tc.tile_pool(name="misc", bufs=2))
        # PSUM pools: aux lives through phases 0+A, closed before phase B
        # (which needs all 8 banks for its accumulators).
        aux_ctx = ExitStack()
        aux_ps = aux_ctx.enter_context(
            tc.tile_pool(name="aux_ps", bufs=1, space="PSUM"))

        # ---- constants -----------------------------------------------------
        ones128f = consts.tile([128, 1], F32)
        nc.vector.memset(ones128f, 1.0)
        ones128 = consts.tile([128, 1], F32R)
        nc.scalar.copy(ones128, ones128f)
        ones1f = consts.tile([1, 128], F32)
        nc.vector.memset(ones1f, 1.0)
        ones1 = consts.tile([1, 128], F32R)
        nc.scalar.copy(ones1, ones1f)
        gq_s = consts.tile([1, QL], F32R)
        nc.sync.dma_start(out=gq_s, in_=t["gq"][:, :].bitcast(F32R))
        gkv_s = consts.tile([1, KV], F32R)
        nc.sync.dma_start(out=gkv_s, in_=t["gkv"][:, :].bitcast(F32R))
        eps_s = consts.tile([1, 1], F32)
        nc.vector.memset(eps_s, EPS)
        rotp_s = consts.tile([128, 128], BF16)
        nc.sync.dma_start(out=rotp_s, in_=t["rotp"][:, :])
        # the query rows are always the first SQ columns of the (per-core
        # rotated) sequence, so the q rope tables are slices of the k ones
        cosq_s = consts.tile([128, SQ], F32)
        nc.sync.dma_start(out=cosq_s, in_=t["cosk"][:, 0:SQ])
        sinq_s = consts.tile([128, SQ], F32)
        nc.sync.dma_start(out=sinq_s, in_=t["sink"][:, 0:SQ])

        # ---- persistent tiles (bf16: matmul operands -> FWL weight loads) --
        kvcT = [persist.tile([128, S], BF16, tag=f"kvcT{c}", name=f"kvcT{c}") for c in range(NKV)]
        krT = persist.tile([128, S], BF16, tag="krT")
        qcT = [persist.tile([128, SQ], BF16, tag=f"qcT{c}", name=f"qcT{c}") for c in range(NQL)]
        oT = [persist.tile([128, SQ], BF16, tag=f"oT{h}", name=f"oT{h}") for h in range(H)]

        # ===================================================================
        # Phase 0: compress. kvcT/krT over full seq, qcT over own query rows.
        # ===================================================================
        with nc.named_scope("p0_compress", notify=True), \
             tc.tile_pool(name="misc0", bufs=2) as misc0, \
             tc.tile_pool(name="xhold", bufs=1) as xholdp, \
             tc.tile_pool(name="wkvhold", bufs=1) as wkvhold, \
             tc.tile_pool(name="wstream", bufs=3) as wstream, \
             tc.tile_pool(name="acc_ps", bufs=6, space="PSUM") as acc_ps:
            # wckv/wkr are reused by all 4 seq blocks: load once, keep in SBUF
            wkv_h = [wkvhold.tile([128, KV], BF16, tag=f"wckv{d}", name=f"wckv{d}")
                     for d in range(16)]
            wkr_h = [wkvhold.tile([128, 128], BF16, tag=f"wkr{d}", name=f"wkr{d}")
                     for d in range(16)]
            # x held in SBUF as 16 full-row tiles: one 4KB-contiguous-line DMA
            # per d-chunk serves all 4 kv blocks AND the q compress (the query
            # block is columns 0:SQ of the rotated sequence).
            xh = [xholdp.tile([128, S], BF16, tag=f"xh{d}", name=f"xh{d}")
                  for d in range(16)]
            for d in range(16):
                drow = slice(d * 128, (d + 1) * 128)
                nc.sync.dma_start(out=wkv_h[d], in_=t["wckv"][drow, :])
                nc.sync.dma_start(out=wkr_h[d], in_=t["wkr2"][drow, :])
                nc.sync.dma_start(out=xh[d], in_=t["xT"][drow, :])
            def p0_post(scol, kvraw, kraw):
                # rmsnorm over kv features (partition dim across the 4 chunks)
                ssq = aux_ps.tile([1, 512], F32, tag="aux")
                for c in range(NKV):
                    sq = misc0.tile([128, 512], F32R, tag="sq")
                    nc.scalar.square(sq, kvraw[c])
                    nc.tensor.matmul(ssq, _r(ones128), _r(sq),
                                     start=(c == 0), stop=(c == NKV - 1))
                rstd = misc0.tile([1, 512], F32R, tag="rstd")
                nc.scalar.activation(rstd, ssq,
                                     mybir.ActivationFunctionType.Sqrt,
                                     bias=eps_s[:, :], scale=1.0 / KV)
                with nc.allow_low_precision(reason="f32r is full fp32 bits"):
                    nc.vector.reciprocal(rstd, rstd)
                for c in range(NKV):
                    bc = aux_ps.tile([128, 512], F32, tag="aux")
                    nc.tensor.matmul(
                        bc, _r(gkv_s[:, c * 128:(c + 1) * 128]), _r(rstd))
                    bc_s = misc.tile([128, 512], F32, tag="bcs")
                    nc.scalar.copy(bc_s, bc)
                    nc.vector.tensor_mul(kvcT[c][:, scol], kvraw[c], bc_s)

                # rope on the (duplicated-rows) k_rope block
                ck = misc0.tile([128, 512], F32, tag="ck")
                nc.sync.dma_start(out=ck, in_=t["cosk"][:, scol])
                sk = misc0.tile([128, 512], F32, tag="sk")
                nc.sync.dma_start(out=sk, in_=t["sink"][:, scol])
                rot = aux_ps.tile([128, 512], F32, tag="aux")
                nc.tensor.matmul(rot, rotp_s, kraw)
                t1 = misc0.tile([128, 512], F32, tag="ropet1")
                nc.vector.tensor_mul(t1, kraw, ck)
                t2 = misc0.tile([128, 512], F32, tag="ropet2")
                nc.vector.tensor_mul(t2, rot, sk)
                nc.vector.tensor_add(krT[:, scol], t1, t2)

            p0_pending = None
            for sb in range(NS):
                scol = slice(sb * 512, (sb + 1) * 512)
                pkv = [acc_ps.tile([128, 512], F32, tag="acc", name="pkv") for _ in range(NKV)]
                pkr = acc_ps.tile([128, 512], F32, tag="acc")
                for d in range(16):
                    for c in range(NKV):
                        nc.tensor.matmul(
                            pkv[c], wkv_h[d][:, c * 128:(c + 1) * 128],
                            xh[d][:, scol], start=(d == 0), stop=(d == 15))
                    nc.tensor.matmul(pkr, wkr_h[d], xh[d][:, scol],
                                     start=(d == 0), stop=(d == 15))
                # drain psum to raw bf16 sbuf tiles (releases acc banks), then
                # run the PREVIOUS block's normalize behind this block's MMs.
                kvraw = [misc0.tile([128, 512], BF16, tag=f"kvraw{c}",
                                    name=f"kvraw{c}")
                         for c in range(NKV)]
                for c in range(NKV):
                    nc.scalar.copy(kvraw[c], pkv[c])
                kraw = misc0.tile([128, 512], BF16, tag="kraw")
                nc.scalar.copy(kraw, pkr)
                if p0_pending is not None:
                    p0_post(*p0_pending)
                p0_pending = (scol, kvraw, kraw)

            # qcT over own query rows (= columns 0:SQ of the rotated seq)
            pqc = [acc_ps.tile([128, 512], F32, tag="acc", name="pqc") for _ in range(NQL)]
            for d in range(16):
                drow = slice(d * 128, (d + 1) * 128)
                wq_t = wstream.tile([128, QL], BF16, tag="wcq")
                nc.sync.dma_start(out=wq_t, in_=t["wcq"][drow, :])
                for c in range(NQL):
                    nc.tensor.matmul(
                        pqc[c], wq_t[:, c * 128:(c + 1) * 128], xh[d][:, 0:SQ],
                        start=(d == 0), stop=(d == 15))
            if p0_pending is not None:
                p0_post(*p0_pending)
                p0_pending = None
            ssq = aux_ps.tile([1, 512], F32, tag="aux")
            for c in range(NQL):
                sq = misc0.tile([128, 512], F32R, tag="sq")
                nc.scalar.square(sq, pqc[c])
                nc.tensor.matmul(ssq, _r(ones128), _r(sq),
                                 start=(c == 0), stop=(c == NQL - 1))
            rstd = misc0.tile([1, 512], F32R, tag="rstd")
            nc.scalar.activation(rstd, ssq, mybir.ActivationFunctionType.Sqrt,
                                 bias=eps_s[:, :], scale=1.0 / QL)
            with nc.allow_low_precision(reason="f32r is full fp32 bits"):
                nc.vector.reciprocal(rstd, rstd)
            for c in range(NQL):
                bc = aux_ps.tile([128, 512], F32, tag="aux")
                nc.tensor.matmul(bc, _r(gq_s[:, c * 128:(c + 1) * 128]), _r(rstd))
                bc_s = misc.tile([128, 512], F32, tag="bcs")
                nc.scalar.copy(bc_s, bc)
                nc.vector.tensor_mul(qcT[c], pqc[c], bc_s)

        # ===================================================================
        # Phase A: per head group -- decompress k/v/q, attention.
        # Scores run as fp8e4 DoubleRow matmuls: contraction slots [p, i]
        # hold nope dims (i=0) and rope dims (i=1, rows 0:64 for even heads /
        # 64:128 for odd heads, zero elsewhere), so one K=256 DR matmul per
        # (head, kt) replaces the K=128 nope + K=64 rope pair. fp8 on the
        # score operands costs ~0.9% rel err on the output (logit errors are
        # shrunk by SCALE before exp; measured in quant_study.py).
        # ===================================================================
        with nc.named_scope("pA_attn", notify=True), \
             tc.tile_pool(name="vpool", bufs=24) as vpool, \
             tc.tile_pool(name="khp", bufs=2) as khp, \
             tc.tile_pool(name="qmp", bufs=2) as qmp, \
             tc.tile_pool(name="ptp", bufs=4) as ptp, \
             tc.tile_pool(name="denp", bufs=2) as denp, \
             tc.tile_pool(name="wdqp", bufs=6) as wdqp, \
             tc.tile_pool(name="wdqrp", bufs=6) as wdqrp, \
             tc.tile_pool(name="wdkp", bufs=4) as wdkp, \
             tc.tile_pool(name="wdvp", bufs=4) as wdvp, \
             tc.tile_pool(name="st_ps", bufs=3, space="PSUM") as st_ps, \
             tc.tile_pool(name="ot_ps", bufs=2, space="PSUM") as ot_ps, \
             tc.tile_pool(name="wk_ps", bufs=2, space="PSUM") as wk_ps:

            # kr-with-zeros fp8 patterns DMAd into each pair's kh[:, 1, :]:
            # krzA rows 0:64 = kr (even head), krzB rows 64:128 = kr (odd).
            krzA = persist.tile([128, S], F8, tag="krzA")
            krzB = persist.tile([128, S], F8, tag="krzB")
            nc.vector.memset(krzA, 0.0)
            nc.vector.memset(krzB, 0.0)
            nc.scalar.copy(krzA[0:64, :], krT[0:64, :])
            nc.scalar.copy(krzB[64:128, :], krT[64:128, :])

            pending = []
            for g in range(GROUPS):
                gcol = slice(g * 512, (g + 1) * 512)
                # stream this group's decompress weights
                wdv_t = [wdvp.tile([128, 512], BF16, tag="wdv", name="wdv_t") for _ in range(NKV)]
                for c in range(NKV):
                    nc.sync.dma_start(
                        out=wdv_t[c], in_=t["wdv"][c * 128:(c + 1) * 128, gcol])
                wdk_t = [wdkp.tile([128, 512], BF16, tag="wdk", name="wdk_t") for _ in range(NKV)]
                for c in range(NKV):
                    nc.sync.dma_start(
                        out=wdk_t[c], in_=t["wdk"][c * 128:(c + 1) * 128, gcol])
                wdq_t = [wdqp.tile([128, 512], BF16, tag="wdq", name="wdq_t") for _ in range(NQL)]
                for c in range(NQL):
                    nc.sync.dma_start(
                        out=wdq_t[c], in_=t["wdq"][c * 128:(c + 1) * 128, gcol])
                grcol = slice(g * 256, (g + 1) * 256)
                wdqr_t = [wdqrp.tile([128, 256], BF16, tag="wdqr", name="wdqr_t") for _ in range(NQL)]
                for c in range(NQL):
                    nc.sync.dma_start(
                        out=wdqr_t[c], in_=t["wdqr"][c * 128:(c + 1) * 128, grcol])

                # v for all 4 heads of the group: moving = wdv (512 wide),
                # stationary = kvc seq-tile. Halves the matmul/LDW count vs
                # the per-pair 256-wide variant.
                vt = {}
                for st in range(NST):
                    pv = wk_ps.tile([128, 512], F32, tag="wk")
                    for c in range(NKV):
                        nc.tensor.matmul(
                            pv, kvcT[c][:, st * 128:(st + 1) * 128], wdv_t[c],
                            start=(c == 0), stop=(c == NKV - 1))
                    v_s = vpool.tile([128, 512], BF16, tag="v")
                    nc.scalar.copy(v_s, pv)
                    vt[st] = v_s

                for pair in range(GH // 2):
                    hA = g * GH + 2 * pair
                    hB = hA + 1
                    colA = slice((2 * pair) * 128, (2 * pair + 1) * 128)
                    colB = slice((2 * pair + 1) * 128, (2 * pair + 2) * 128)

                    # k^T DoubleRow tiles for both heads: [128, 2, S] fp8
                    # (i=0 nope from decompress, i=1 rope pattern via DMA)
                    khA = khp.tile([128, 2, S], F8, tag="kh")
                    khB = khp.tile([128, 2, S], F8, tag="kh")
                    nc.sync.dma_start(out=khA[:, 1, :], in_=krzA[:, :])
                    nc.sync.dma_start(out=khB[:, 1, :], in_=krzB[:, :])
                    for kh, hcol in ((khA, colA), (khB, colB)):
                        for blk in range(NS):
                            bcol = slice(blk * 512, (blk + 1) * 512)
                            pk = wk_ps.tile([128, 512], F32, tag="wk")
                            for c in range(NKV):
                                nc.tensor.matmul(
                                    pk, wdk_t[c][:, hcol], kvcT[c][:, bcol],
                                    start=(c == 0), stop=(c == NKV - 1))
                            nc.scalar.copy(kh[:, 0, bcol], pk)

                    # q DoubleRow tiles for both heads: [128, 2, SQ] fp8
                    # (i=0 nope, i=1 roped q_rope -- both heads' rope rows,
                    # masked by the zeros in the kh rope pattern)
                    qmA = qmp.tile([128, 2, SQ], F8, tag="qm")
                    qmB = qmp.tile([128, 2, SQ], F8, tag="qm")
                    for qm, hcol in ((qmA, colA), (qmB, colB)):
                        pq = wk_ps.tile([128, SQ], F32, tag="wk")
                        for c in range(NQL):
                            nc.tensor.matmul(pq, wdq_t[c][:, hcol], qcT[c],
                                             start=(c == 0), stop=(c == NQL - 1))
                        nc.scalar.copy(qm[:, 0, :], pq)

                    # normalize the PREVIOUS pair, stage 1 (its den DVE
                    # chain drained behind the decompress matmuls above)
                    recs = _norm_den(nc, misc, st_ps, ones128, pending)

                    # q_rope for the pair (two heads stacked on partitions)
                    prcol = slice(pair * 128, (pair + 1) * 128)
                    pqr = wk_ps.tile([128, SQ], F32, tag="wk")
                    for c in range(NQL):
                        nc.tensor.matmul(
                            pqr, wdqr_t[c][:, prcol], qcT[c],
                            start=(c == 0), stop=(c == NQL - 1))
                    qraw = misc.tile([128, SQ], BF16, tag="qraw")
                    nc.scalar.copy(qraw, pqr)
                    rot = aux_ps.tile([128, SQ], F32, tag="aux")
                    nc.tensor.matmul(rot, rotp_s, qraw)
                    t1 = misc.tile([128, SQ], F32, tag="ropet1")
                    nc.vector.tensor_mul(t1, qraw, cosq_s)
                    t2 = misc.tile([128, SQ], F32, tag="ropet2")
                    nc.vector.tensor_mul(t2, rot, sinq_s)
                    nc.vector.tensor_add(qmA[:, 1, :], t1, t2)
                    nc.vector.tensor_add(qmB[:, 1, :], t1, t2)

                    # ...stage 2 lands after the rope matmuls so the DVE
                    # reciprocal latency is PE-covered.
                    _norm_fin(nc, misc, st_ps, oT, ones1, pending, recs)
                    pending.clear()

                    # attention for the pair: one fp8 DoubleRow matmul per
                    # (head, kt) covers nope + rope contraction.
                    potA = ot_ps.tile([128, SQ], F32, tag="ot")
                    potB = ot_ps.tile([128, SQ], F32, tag="ot")
                    denA = denp.tile([128, SQ], F32R, tag="den")
                    denB = denp.tile([128, SQ], F32R, tag="den")
                    for kt in range(NST):
                        kcol = slice(kt * 128, (kt + 1) * 128)
                        pstA = st_ps.tile([128, SQ], F32, tag="st")
                        pstB = st_ps.tile([128, SQ], F32, tag="st")
                        nc.tensor.matmul(pstA, khA[:, :, kcol], qmA[:, :, :],
                                         perf_mode=DR, start=True, stop=True)
                        nc.tensor.matmul(pstB, khB[:, :, kcol], qmB[:, :, :],
                                         perf_mode=DR, start=True, stop=True)
                        ptA = ptp.tile([128, SQ], BF16, tag="pt")
                        nc.scalar.activation(ptA, pstA,
                                             mybir.ActivationFunctionType.Exp,
                                             scale=SCALE)
                        ptB = ptp.tile([128, SQ], BF16, tag="pt")
                        nc.scalar.activation(ptB, pstB,
                                             mybir.ActivationFunctionType.Exp,
                                             scale=SCALE)
                        if kt == 0:
                            nc.vector.tensor_copy(denA, ptA)
                            nc.vector.tensor_copy(denB, ptB)
                        else:
                            nc.vector.tensor_add(denA, denA, ptA)
                            nc.vector.tensor_add(denB, denB, ptB)
                        vs = vt[kt]
                        nc.tensor.matmul(
                            potA, vs[:, colA], ptA,
                            start=(kt == 0), stop=(kt == NST - 1))
                        nc.tensor.matmul(
                            potB, vs[:, colB], ptB,
                            start=(kt == 0), stop=(kt == NST - 1))

                    pending.append((hA, potA, denA))
                    pending.append((hB, potB, denB))

            recs = _norm_den(nc, misc, st_ps, ones128, pending)
            _norm_fin(nc, misc, st_ps, oT, ones1, pending, recs)
            pending.clear()

        if debug:
            for c in range(NKV):
                nc.sync.dma_start(
                    out=t["dbg_kvcT"][c * 128:(c + 1) * 128, :], in_=kvcT[c])
            for c in range(NQL):
                nc.sync.dma_start(
                    out=t["dbg_qcT"][c * 128:(c + 1) * 128, :], in_=qcT[c])
            nc.sync.dma_start(out=t["dbg_krT"][:, :], in_=krT)
            for h in range(H):
                nc.sync.dma_start(
                    out=t["dbg_oT"][h * 128:(h + 1) * 128, :], in_=oT[h])

        # ===================================================================
        # Phase B: output projection, all 16 heads, PSUM-accumulated.
        # h-outer over D-halves: wo streams as 16 [128, 1024] row tiles per
        # half (2KB contiguous DMA lines, one DMA per head) while the 8 PSUM
        # banks hold one half's accumulators; each stationary oT slice is
        # loaded once per (half, qt) and serves both 512-wide D blocks.
        # ===================================================================
        aux_ctx.close()
        NQT = SQ // 128
        with nc.named_scope("pB_outproj", notify=True), \
             tc.tile_pool(name="wop", bufs=6) as wop, \
             tc.tile_pool(name="outs", bufs=4) as outs, \
             tc.tile_pool(name="po_ps", bufs=8, space="PSUM") as po_ps:
            for half in range(2):
                hcol = slice(half * 1024, (half + 1) * 1024)
                po = [[po_ps.tile([128, 512], F32, tag="po", name=f"po{b2}_{qt}")
                       for qt in range(NQT)] for b2 in range(2)]
                for h in range(H):
                    wo_t = wop.tile([128, 1024], BF16, tag="wo")
                    nc.sync.dma_start(
                        out=wo_t, in_=t["wo"][h * 128:(h + 1) * 128, hcol])
                    for qt in range(NQT):
                        for b2 in range(2):
                            nc.tensor.matmul(
                                po[b2][qt], oT[h][:, qt * 128:(qt + 1) * 128],
                                wo_t[:, b2 * 512:(b2 + 1) * 512],
                                start=(h == 0), stop=(h == H - 1))
                for b2 in range(2):
                    bcol = slice(half * 1024 + b2 * 512,
                                 half * 1024 + (b2 + 1) * 512)
                    for qt in range(NQT):
                        o_s = outs.tile([128, 512], F32, tag="os")
                        nc.scalar.copy(o_s, po[b2][qt])
                        nc.sync.dma_start(
                            out=t["out"][qt * 128:(qt + 1) * 128, bcol], in_=o_s)


_NC_CACHE = None


def _get_nc():
    global _NC_CACHE
    if _NC_CACHE is None:
        _NC_CACHE = build_nc()
    return _NC_CACHE


def _rope_tables(positions):
    """cos/sin tables in transposed-packed layout [128, len(positions)]:
    rows 0:64 and 64:128 both hold the [RD, s] table (two rope vectors are
    stacked per 128 partitions)."""
    inv_freq = 1.0 / (10000.0 ** (np.arange(0, RD, 2, dtype=np.float32) / RD))
    ang = positions[:, None].astype(np.float32) * inv_freq[None, :]  # [s, 32]
    cos = np.concatenate([np.cos(ang), np.cos(ang)], axis=-1)        # [s, 64]
    sin = np.concatenate([np.sin(ang), np.sin(ang)], axis=-1)
    cosT = np.ascontiguousarray(cos.T)                               # [64, s]
    sinT = np.ascontiguousarray(sin.T)
    return (np.concatenate([cosT, cosT], axis=0),
            np.concatenate([sinT, sinT], axis=0))


def _rot_perm():
    m = np.zeros((128, 128), dtype=np.float32)
    for b0 in (0, 64):
        for i in range(32):
            m[b0 + i + 32, b0 + i] = -1.0   # rot[m] = -t[m+32], m < 32
            m[b0 + i, b0 + i + 32] = 1.0    # rot[m] = +t[m-32], m >= 32
    return m


def kernel(x, Wcq, g_q, Wdq, Wdqr, Wckv, g_kv, Wdk, Wdv, Wkr, Wo):
    import ml_dtypes

    bf16 = ml_dtypes.bfloat16
    nc = _get_nc()

    x = np.asarray(x, dtype=np.float32)
    xT = [np.ascontiguousarray(x[b].T).astype(bf16) for b in range(B)]  # [D, S]
    wkr2 = np.ascontiguousarray(
        np.concatenate([Wkr, Wkr], axis=1)).astype(bf16)  # [D, 128]
    rotp = _rot_perm().astype(bf16)

    shared = {
        "wcq": np.ascontiguousarray(Wcq).astype(bf16),
        "wckv": np.ascontiguousarray(Wckv).astype(bf16),
        "wkr2": wkr2,
        "wdq": np.ascontiguousarray(Wdq).astype(bf16),
        "wdqr": np.ascontiguousarray(Wdqr).astype(bf16),
        "wdk": np.ascontiguousarray(Wdk).astype(bf16),
        "wdv": np.ascontiguousarray(Wdv).astype(bf16),
        "wo": np.ascontiguousarray(Wo).astype(bf16),
        "gq": np.ascontiguousarray(g_q, dtype=np.float32).reshape(1, QL),
        "gkv": np.ascontiguousarray(g_kv, dtype=np.float32).reshape(1, KV),
        "rotp": rotp,
    }

    # Each core sees the sequence rotated so its own query block sits at
    # columns 0:SQ (the SPMD program is position-independent; softmax over
    # keys is permutation invariant as long as the rope tables rotate too).
    in_maps = []
    for core in range(N_CORES):
        b, sl = core // 4, core % 4
        pos = np.roll(np.arange(S), -sl * SQ)
        ck, sk = _rope_tables(pos)
        m = dict(shared)
        m["xT"] = np.ascontiguousarray(np.roll(xT[b], -sl * SQ, axis=1))
        m["cosk"] = np.ascontiguousarray(ck)
        m["sink"] = np.ascontiguousarray(sk)
        in_maps.append(m)

    trace = bool(int(os.environ.get("MLA_TRACE", "0")))
    res = run_bass_kernel_spmd(
        nc, in_maps, core_ids=list(range(N_CORES)), trace=trace,
        trace_cores=list(range(N_CORES)) if trace else None,
        stitch_traces=bool(int(os.environ.get("MLA_STITCH", "0"))),
        tmpdir=os.environ.get("MLA_TMPDIR") or None,
    )
    kernel.last_result = res

    out = np.empty((B, S, D), dtype=np.float32)
    for core in range(N_CORES):
        b, sl = core // 4, core % 4
        out[b, sl * SQ:(sl + 1) * SQ, :] = res.results[core]["out"]
    return out

